# revision 11
# baseline (speedup 1.0000x reference)
"""Trainium2 Bass kernel for nn_DepthModule: ray-marched implicit-surface depth.

kernel(**inputs) takes FULL unsharded inputs (ray0 [1,8192,3], ray_direction
[1,8192,3], MLP weights W1..b4) and returns the FULL output [1,8192] float32.
The N=8192 ray axis is sharded across 8 NeuronCores (data parallel, weights
replicated); each core runs an identical Bass/Tile program on its 1024 rays.

Per-core pipeline (mirrors the jax reference):
  1. cube entry/exit depths per ray (DVE vector math, rays on partitions)
  2. S=128 proposal depths/ray -> MLP occupancy logits for all 128K points
     (fp32r matmuls on PE; softplus(z) = Ln(Exp(z)+1) on the scalar engine,
     biases folded into Exp's pre-activation bias)
  3. first-sign-change scan (one-hot gather via tensor_tensor_reduce)
  4. 8 secant-refinement iterations (1024-point MLP evals)
  5. implicit-gradient Newton correction (forward + JVP tangent pass;
     sigmoid(z) = E/(E+1) recovered from the saved Exp values on DVE)
"""

from contextlib import ExitStack

import numpy as np

import concourse.bacc as bacc
import concourse.bass as bass
import concourse.tile as tile
import concourse.mybir as mybir
from concourse.bass_utils import run_bass_kernel_spmd

F32 = mybir.dt.float32
F32R = mybir.dt.float32r
U32 = mybir.dt.uint32
AF = mybir.ActivationFunctionType
ALU = mybir.AluOpType
AX = mybir.AxisListType

NCORES = 8
N = 8192
R = N // NCORES          # rays per core = 1024
S = 128                  # proposal samples per ray
H = 256                  # MLP hidden dim
NSEC = 8                 # secant iterations
NTILE = R // 128         # ray tiles per core = 8
BP = 1024                # points per stage-A batch (8 rays x 128 samples)
NB = R * S // BP         # stage-A batches = 128

TAU = 0.5
LOGIT_TAU = float(np.log(TAU / (1.0 - TAU)))   # 0.0
DEPTH_LO, DEPTH_HI = 0.0, 2.4
PADDING = 0.1
EPS = 1e-6
P_DIST = 0.5 + PADDING / 2.0
BIG = 1e9

_nc_cache = {}
import os
STOP_AFTER = os.environ.get("K_STOP_AFTER", "full")
# InstTensorTensorReduce crashes the exec unit on this toolchain/HW
USE_TTR = os.environ.get("K_USE_TTR", "0") == "1"


def expand_ap(ap, pattern, offset_elems=0):
    """AP over `ap`'s tensor keeping its partition dim, with explicit free
    [stride, count] pairs (strides in elements, outer->inner)."""
    new = [ap.ap[0]] + [[s, c] for (s, c) in pattern]
    return bass.AP(tensor=ap.tensor, offset=ap.offset + offset_elems, ap=new)


def build_program():
    nc = bacc.Bacc("TRN2", target_bir_lowering=False, debug=False,
                   num_devices=NCORES)

    d_ray0 = nc.dram_tensor("ray0", [R, 3], F32, kind="ExternalInput").ap()
    d_rd = nc.dram_tensor("rd", [R, 3], F32, kind="ExternalInput").ap()
    d_W1 = nc.dram_tensor("W1", [3, H], F32, kind="ExternalInput").ap()
    d_b1 = nc.dram_tensor("b1", [H], F32, kind="ExternalInput").ap()
    d_W2 = nc.dram_tensor("W2", [H, H], F32, kind="ExternalInput").ap()
    d_b2 = nc.dram_tensor("b2", [H], F32, kind="ExternalInput").ap()
    d_W3 = nc.dram_tensor("W3", [H, H], F32, kind="ExternalInput").ap()
    d_b3 = nc.dram_tensor("b3", [H], F32, kind="ExternalInput").ap()
    d_W4 = nc.dram_tensor("W4", [H, 1], F32, kind="ExternalInput").ap()
    d_b4 = nc.dram_tensor("b4", [1], F32, kind="ExternalInput").ap()
    d_t128 = nc.dram_tensor("t128", [128, S], F32, kind="ExternalInput").ap()
    d_wcost = nc.dram_tensor("wcost", [128, S], F32, kind="ExternalInput").ap()
    d_ident = nc.dram_tensor("ident", [128, 128], F32, kind="ExternalInput").ap()
    d_out = nc.dram_tensor("out", [R], F32, kind="ExternalOutput").ap()

    with tile.TileContext(nc) as tc:
        _emit(nc, tc, d_ray0, d_rd, d_W1, d_b1, d_W2, d_b2, d_W3, d_b3,
              d_W4, d_b4, d_t128, d_wcost, d_ident, d_out)

    nc.compile()
    return nc


def _emit(nc, tc, d_ray0, d_rd, d_W1, d_b1, d_W2, d_b2, d_W3, d_b3,
          d_W4, d_b4, d_t128, d_wcost, d_ident, d_out):
    ctx = ExitStack()
    sing = ctx.enter_context(tc.tile_pool(name="sing", bufs=1))
    work = ctx.enter_context(tc.tile_pool(name="work", bufs=2))
    z1p = ctx.enter_context(tc.tile_pool(name="z1p", bufs=2))
    ep = ctx.enter_context(tc.tile_pool(name="ep", bufs=2))
    fin = ctx.enter_context(tc.tile_pool(name="fin", bufs=1))
    bcp = ctx.enter_context(tc.tile_pool(name="bcp", bufs=2))
    rowp = ctx.enter_context(tc.tile_pool(name="rowp", bufs=2))
    psA = ctx.enter_context(tc.tile_pool(name="psA", bufs=3, space="PSUM"))
    psZ4 = ctx.enter_context(tc.tile_pool(name="psZ4", bufs=1, space="PSUM"))
    dram = ctx.enter_context(tc.tile_pool(name="dram", bufs=1, space="DRAM"))

    # ================= constants & weights =================
    t128 = sing.tile([128, S], F32)        # t128[p, s] = t[s]
    wcost = sing.tile([128, S], F32)       # sign-scan cost weights
    ident = sing.tile([128, 128], F32)
    nc.sync.dma_start(out=t128[:], in_=d_t128)
    nc.sync.dma_start(out=wcost[:], in_=d_wcost)
    nc.sync.dma_start(out=ident[:], in_=d_ident)

    w2 = [[sing.tile([128, 128], F32R, tag=f"w2_{k}_{c}", name=f"w2_{k}_{c}")
           for c in range(2)] for k in range(2)]
    w3 = [[sing.tile([128, 128], F32R, tag=f"w3_{k}_{c}", name=f"w3_{k}_{c}")
           for c in range(2)] for k in range(2)]
    w4 = [sing.tile([128, 1], F32R, tag=f"w4_{k}", name=f"w4_{k}")
          for k in range(2)]
    w1 = [sing.tile([3, 128], F32R, tag=f"w1_{c}", name=f"w1_{c}")
          for c in range(2)]
    for k in range(2):
        for c in range(2):
            nc.sync.dma_start(out=w2[k][c][:],
                              in_=d_W2[k * 128:(k + 1) * 128,
                                       c * 128:(c + 1) * 128].bitcast(F32R))
            nc.sync.dma_start(out=w3[k][c][:],
                              in_=d_W3[k * 128:(k + 1) * 128,
                                       c * 128:(c + 1) * 128].bitcast(F32R))
        nc.sync.dma_start(out=w4[k][:],
                          in_=d_W4[k * 128:(k + 1) * 128, :].bitcast(F32R))
    for c in range(2):
        nc.sync.dma_start(out=w1[c][:],
                          in_=d_W1[:, c * 128:(c + 1) * 128].bitcast(F32R))

    b1c = [sing.tile([128, 1], F32, tag=f"b1_{c}", name=f"b1_{c}") for c in range(2)]
    b2c = [sing.tile([128, 1], F32, tag=f"b2_{c}", name=f"b2_{c}") for c in range(2)]
    b3c = [sing.tile([128, 1], F32, tag=f"b3_{c}", name=f"b3_{c}") for c in range(2)]
    for c in range(2):
        nc.sync.dma_start(out=b1c[c][:], in_=d_b1[c * 128:(c + 1) * 128]
                          .rearrange("(h o) -> h o", o=1))
        nc.sync.dma_start(out=b2c[c][:], in_=d_b2[c * 128:(c + 1) * 128]
                          .rearrange("(h o) -> h o", o=1))
        nc.sync.dma_start(out=b3c[c][:], in_=d_b3[c * 128:(c + 1) * 128]
                          .rearrange("(h o) -> h o", o=1))
    b4col = sing.tile([128, 1], F32)
    nc.sync.dma_start(out=b4col[:],
                      in_=bass.AP(tensor=d_b4.tensor, offset=d_b4.offset,
                                  ap=[[0, 128], [1, 1]]))

    r0T = sing.tile([3, R], F32R)
    rdT = sing.tile([3, R], F32R)
    nc.sync.dma_start(out=r0T[:], in_=d_ray0.rearrange("n c -> c n").bitcast(F32R))
    nc.sync.dma_start(out=rdT[:], in_=d_rd.rearrange("n c -> c n").bitcast(F32R))

    # ================= A/C layer-1 folds =================
    # A[c][f, ray] = (W1^T ray0^T)[f, ray] + b1[f];  C[c] = W1^T rd^T
    A = [sing.tile([128, R], F32, tag=f"A_{c}", name=f"A_{c}") for c in range(2)]
    C = [sing.tile([128, R], F32, tag=f"C_{c}", name=f"C_{c}") for c in range(2)]
    for c in range(2):
        for h in range(2):
            ps = psA.tile([128, BP], F32, tag="z")
            nc.tensor.matmul(ps[:, 0:512], w1[c][:],
                             r0T[:, h * 512:(h + 1) * 512],
                             start=True, stop=True)
            nc.scalar.activation(A[c][:, h * 512:(h + 1) * 512], ps[:, 0:512],
                                 AF.Identity, bias=b1c[c][:])
            ps2 = psA.tile([128, BP], F32, tag="z")
            nc.tensor.matmul(ps2[:, 0:512], w1[c][:],
                             rdT[:, h * 512:(h + 1) * 512],
                             start=True, stop=True)
            nc.scalar.activation(C[c][:, h * 512:(h + 1) * 512], ps2[:, 0:512],
                                 AF.Identity)

    # ================= cube intersection =================
    statsd = dram.tile([NTILE, 2, 128], F32)   # [tile][dn_sel; g_sel][ray]
    bigt = sing.tile([128, 6], F32)
    nc.vector.memset(bigt[:], BIG)
    c1e9 = sing.tile([128, 3], F32)
    nc.vector.memset(c1e9[:], 1e-9)
    c1em12 = sing.tile([128, NTILE], F32)
    nc.vector.memset(c1em12[:], 1e-12)

    for p in range(NTILE):
        r0t = work.tile([128, 3], F32, tag="r0t")
        rdt = work.tile([128, 3], F32, tag="rdt")
        nc.sync.dma_start(out=r0t[:], in_=d_ray0[p * 128:(p + 1) * 128, :])
        nc.sync.dma_start(out=rdt[:], in_=d_rd[p * 128:(p + 1) * 128, :])

        # den = where(|rd| < 1e-9, 1e-9, rd)
        den = work.tile([128, 3], F32, tag="den")
        nc.vector.tensor_scalar(den[:].bitcast(U32), rdt[:].bitcast(U32),
                                0x7FFFFFFF, None, ALU.bitwise_and)
        msmall = work.tile([128, 3], F32, tag="msmall")
        nc.vector.tensor_scalar(msmall[:], den[:], 1e-9, None, ALU.is_lt)
        nc.vector.tensor_copy(den[:], rdt[:])
        nc.vector.copy_predicated(den[:], msmall[:].bitcast(U32), c1e9[:])
        inv = work.tile([128, 3], F32, tag="inv")
        nc.vector.reciprocal(inv[:], den[:])

        # d6 = (plane - ray0) * inv, planes (+p,+p,+p,-p,-p,-p)
        d6 = work.tile([128, 6], F32, tag="d6")
        nom = work.tile([128, 3], F32, tag="nom")
        nc.vector.tensor_scalar(nom[:], r0t[:], -1.0, P_DIST, ALU.mult, ALU.add)
        nc.vector.tensor_mul(d6[:, 0:3], nom[:], inv[:])
        nc.vector.tensor_scalar(nom[:], r0t[:], -1.0, -P_DIST, ALU.mult, ALU.add)
        nc.vector.tensor_mul(d6[:, 3:6], nom[:], inv[:])

        # p_int[r, k, j] = ray0[r, j] + d6[r, k] * rd[r, j]
        pi = work.tile([128, 18], F32, tag="pi")
        nc.vector.tensor_tensor(pi[:], expand_ap(d6[:], [(1, 6), (0, 3)]),
                                expand_ap(rdt[:], [(0, 6), (1, 3)]), ALU.mult)
        nc.vector.tensor_tensor(pi[:], pi[:],
                                expand_ap(r0t[:], [(0, 6), (1, 3)]), ALU.add)
        nc.vector.tensor_scalar(pi[:].bitcast(U32), pi[:].bitcast(U32),
                                0x7FFFFFFF, None, ALU.bitwise_and)
        nc.vector.tensor_scalar(pi[:], pi[:], P_DIST + EPS, None, ALU.is_le)
        in6 = work.tile([128, 6], F32, tag="in6")
        nc.vector.tensor_reduce(in6[:], pi[:].rearrange("p (k j) -> p k j", j=3),
                                axis=AX.X, op=ALU.min)
        cnt = work.tile([128, 1], F32, tag="cnt")
        nc.vector.tensor_reduce(cnt[:], in6[:], axis=AX.X, op=ALU.add)
        mcube = work.tile([128, 1], F32, tag="mcube")
        nc.vector.tensor_scalar(mcube[:], cnt[:], 2.0, None, ALU.is_equal)

        # dn = min inside |d|, df = max inside |d|
        nc.vector.tensor_scalar(d6[:].bitcast(U32), d6[:].bitcast(U32),
                                0x7FFFFFFF, None, ALU.bitwise_and)
        lo6 = work.tile([128, 6], F32, tag="lo6")
        nc.vector.tensor_copy(lo6[:], bigt[:])
        nc.vector.copy_predicated(lo6[:], in6[:].bitcast(U32), d6[:])
        dn_r = work.tile([128, 1], F32, tag="dn_r")
        nc.vector.tensor_reduce(dn_r[:], lo6[:], axis=AX.X, op=ALU.min)
        hi6 = work.tile([128, 6], F32, tag="hi6")
        nc.vector.tensor_mul(hi6[:], in6[:], d6[:])
        df_r = work.tile([128, 1], F32, tag="df_r")
        nc.vector.tensor_reduce(df_r[:], hi6[:], axis=AX.X, op=ALU.max)

        # st2 = [mcube*dn, mcube*(df-dn-2.4)+2.4]
        st2 = work.tile([128, 2], F32, tag="st2")
        g_r = work.tile([128, 1], F32, tag="g_r")
        nc.vector.tensor_sub(g_r[:], df_r[:], dn_r[:])
        nc.vector.tensor_mul(st2[:, 0:1], mcube[:], dn_r[:])
        nc.vector.tensor_scalar(g_r[:], g_r[:], 1.0, -(DEPTH_HI - DEPTH_LO),
                                ALU.mult, ALU.add)
        nc.vector.tensor_mul(g_r[:], g_r[:], mcube[:])
        nc.vector.tensor_scalar(st2[:, 1:2], g_r[:], 1.0, DEPTH_HI - DEPTH_LO,
                                ALU.mult, ALU.add)

        pst = psZ4.tile([128, BP], F32, tag="z4")
        nc.tensor.transpose(pst[0:2, 0:128], st2[:], ident[:])
        strow = work.tile([2, 128], F32, tag="strow")
        nc.vector.tensor_copy(strow[:], pst[0:2, 0:128])
        nc.sync.dma_start(out=statsd[p, :, :], in_=strow[:])

    # (tile,k)-partition layouts for the proposal-depth construction
    dn128 = sing.tile([128, NTILE], F32)
    g128 = sing.tile([128, NTILE], F32)
    for p in range(NTILE):
        nc.sync.dma_start(out=dn128[p * 16:(p + 1) * 16, :],
                          in_=statsd[p, 0, :].rearrange("(k r) -> k r", r=8))
        nc.sync.dma_start(out=g128[p * 16:(p + 1) * 16, :],
                          in_=statsd[p, 1, :].rearrange("(k r) -> k r", r=8))
    # (ray,tile) layouts for the scan / secant math
    st_dnc = sing.tile([128, NTILE], F32)
    st_gc = sing.tile([128, NTILE], F32)
    nc.sync.dma_start(out=st_dnc[:], in_=statsd[:, 0, :].rearrange("p r -> r p"))
    nc.sync.dma_start(out=st_gc[:], in_=statsd[:, 1, :].rearrange("p r -> r p"))

    if STOP_AFTER == "cube":
        nc.sync.dma_start(out=d_out.rearrange("(t r) -> r t", r=128),
                          in_=st_dnc[:])
        ctx.close()
        return

    # ================= proposal depths D8b =================
    # D8b[(p,k), (r8, s)] = dn128[(p,k), r8] + t[s] * g128[(p,k), r8]
    D8b = sing.tile([128, BP], F32)
    nc.vector.tensor_tensor(D8b[:], expand_ap(t128[:], [(0, 8), (1, S)]),
                            expand_ap(g128[:], [(1, 8), (0, S)]), ALU.mult)
    nc.vector.tensor_tensor(D8b[:], D8b[:],
                            expand_ap(dn128[:], [(1, 8), (0, S)]), ALU.add)

    vald = dram.tile([NB, BP], F32)     # stage-A logits, flat batch rows
    d8d = dram.tile([NB, BP], F32)      # proposal depths, flat batch rows
    nc.sync.dma_start(out=d8d[:], in_=D8b[:])

    # ================= MLP forward for one 1024-point batch ==============
    def mlp_batch(db_ap, per_ray, want_sig, tag_final=False):
        """db_ap: [128, 1024] depth broadcast. per_ray=(p0,k0) selects
        stage-A per-ray scalars; None means point index == ray index.
        Returns (z4 psum tile, e_list or None)."""
        z1 = z1p.tile([128, 2 * BP], F32, tag="z1")
        if per_ray is not None:
            p0, k0 = per_ray
            g0 = p0 * 128 + k0 * 8
            for c in range(2):
                for j in range(8):
                    nc.vector.tensor_scalar(
                        z1[:, c * BP + j * 128:c * BP + (j + 1) * 128],
                        db_ap[:, j * 128:(j + 1) * 128],
                        C[c][:, g0 + j:g0 + j + 1],
                        A[c][:, g0 + j:g0 + j + 1],
                        ALU.mult, ALU.add)
        else:
            for c in range(2):
                nc.vector.tensor_mul(z1[:, c * BP:(c + 1) * BP], db_ap, C[c][:])
                nc.vector.tensor_add(z1[:, c * BP:(c + 1) * BP],
                                     z1[:, c * BP:(c + 1) * BP], A[c][:])

        if want_sig:
            e1 = fin.tile([128, 2 * BP], F32, tag="e1f", name="e1f")
        else:
            e1 = ep.tile([128, 2 * BP], F32, tag="e", name="e1")
        nc.scalar.activation(e1[:], z1[:], AF.Exp)
        if want_sig:
            h1 = fin.tile([128, 2 * BP], F32R, tag="h1f", name="h1f")
        else:
            h1 = ep.tile([128, 2 * BP], F32R, tag="h", name="h1")
        nc.scalar.activation(h1[:], e1[:], AF.Ln, bias=1.0)
        es = [e1]

        def layer(h_in, wk, bk, tagl):
            if want_sig:
                e = fin.tile([128, 2 * BP], F32, tag="e" + tagl + "f",
                             name="e" + tagl + "f")
            else:
                e = ep.tile([128, 2 * BP], F32, tag="e", name="e" + tagl)
            for c in range(2):
                ps = psA.tile([128, BP], F32, tag="z")
                for k in range(2):
                    for hh in range(2):
                        nc.tensor.matmul(
                            ps[:, hh * 512:(hh + 1) * 512], wk[k][c][:],
                            h_in[:, k * BP + hh * 512:k * BP + (hh + 1) * 512],
                            start=(k == 0), stop=(k == 1))
                nc.scalar.activation(e[:, c * BP:(c + 1) * BP], ps[:],
                                     AF.Exp, bias=bk[c][:])
            if want_sig:
                h = fin.tile([128, 2 * BP], F32R, tag="h" + tagl + "f",
                             name="h" + tagl + "f")
            else:
                h = ep.tile([128, 2 * BP], F32R, tag="h", name="h" + tagl)
            nc.scalar.activation(h[:], e[:], AF.Ln, bias=1.0)
            es.append(e)
            return h

        h2 = layer(h1, w2, b2c, "2")
        h3 = layer(h2, w3, b3c, "3")
        z4 = psZ4.tile([128, BP], F32, tag="z4")
        for k in range(2):
            for hh in range(2):
                nc.tensor.matmul(z4[0:1, hh * 512:(hh + 1) * 512], w4[k][:],
                                 h3[:, k * BP + hh * 512:k * BP + (hh + 1) * 512],
                                 start=(k == 0), stop=(k == 1))
        return z4, (es if want_sig else None)

    def z4_to_dram(z4, dst_row_ap):
        row = rowp.tile([1, BP], F32, tag="z4row")
        nc.vector.tensor_copy(row[:], z4[0:1, :])
        nc.sync.dma_start(out=dst_row_ap, in_=row[:])

    # ================= stage A =================
    for b in range(NB):
        p0, k0 = b // 16, b % 16
        drow = rowp.tile([1, BP], F32, tag="drow")
        nc.sync.dma_start(out=drow[:], in_=d8d[b:b + 1, :])
        db = bcp.tile([128, BP], F32, tag="db")
        nc.gpsimd.partition_broadcast(db[:], drow[:])
        z4, _ = mlp_batch(db[:], (p0, k0), False)
        z4_to_dram(z4, vald[b:b + 1, :])

    if STOP_AFTER == "stageA":
        tmpo = work.tile([128, NTILE], F32, tag="tmpo")
        for p in range(NTILE):
            nc.sync.dma_start(
                out=tmpo[:, p:p + 1],
                in_=vald[p * 16:(p + 1) * 16, :].rearrange(
                    "k (r s) -> (k r) s", s=S)[:, 0:1])
        nc.sync.dma_start(out=d_out.rearrange("(t r) -> r t", r=128),
                          in_=tmpo[:])
        ctx.close()
        return

    # ================= scan =================
    st_dlow = sing.tile([128, NTILE], F32)
    st_flow = sing.tile([128, NTILE], F32)
    st_dhigh = sing.tile([128, NTILE], F32)
    st_fhigh = sing.tile([128, NTILE], F32)
    st_mask = sing.tile([128, NTILE], F32)

    for p in range(NTILE):
        val = work.tile([128, S], F32, tag="val")
        nc.sync.dma_start(
            out=val[:],
            in_=vald[p * 16:(p + 1) * 16, :].rearrange(
                "k (r s) -> (k r) s", s=S))
        nc.vector.tensor_scalar(val[:], val[:], b4col[:], -LOGIT_TAU,
                                ALU.add, ALU.add)

        prod = work.tile([128, S], F32, tag="prod")
        nc.vector.tensor_tensor(prod[:, 0:S - 1], val[:, 0:S - 1],
                                val[:, 1:S], ALU.mult)
        sgn = work.tile([128, S], F32, tag="sgn")
        nc.scalar.activation(sgn[:, 0:S - 1], prod[:, 0:S - 1], AF.Sign)
        nc.vector.memset(sgn[:, S - 1:S], 1.0)
        cost = work.tile([128, S], F32, tag="cost")
        nc.vector.tensor_tensor(cost[:], sgn[:], wcost[:], ALU.mult)
        vmin = work.tile([128, 1], F32, tag="vmin")
        nc.vector.tensor_reduce(vmin[:], cost[:], axis=AX.X, op=ALU.min)
        oh = work.tile([128, S], F32, tag="oh")
        nc.vector.tensor_scalar(oh[:], cost[:], vmin[:], None, ALU.is_equal)
        ohh = work.tile([128, S], F32, tag="ohh")
        nc.vector.memset(ohh[:, 0:1], 0.0)
        nc.vector.tensor_copy(ohh[:, 1:S], oh[:, 0:S - 1])
        nc.vector.tensor_add(ohh[:, S - 1:S], ohh[:, S - 1:S], oh[:, S - 1:S])

        scr = work.tile([128, S], F32, tag="scr")
        f_low = work.tile([128, 1], F32, tag="f_low")
        f_high = work.tile([128, 1], F32, tag="f_high")
        t_low = work.tile([128, 1], F32, tag="t_low")
        t_high = work.tile([128, 1], F32, tag="t_high")
        if USE_TTR:
            nc.vector.tensor_tensor_reduce(scr[:], val[:], oh[:], 1.0, 0.0,
                                           ALU.mult, ALU.add, f_low[:])
            nc.vector.tensor_tensor_reduce(scr[:], val[:], ohh[:], 1.0, 0.0,
                                           ALU.mult, ALU.add, f_high[:])
            nc.vector.tensor_tensor_reduce(scr[:], t128[:], oh[:], 1.0, 0.0,
                                           ALU.mult, ALU.add, t_low[:])
            nc.vector.tensor_tensor_reduce(scr[:], t128[:], ohh[:], 1.0, 0.0,
                                           ALU.mult, ALU.add, t_high[:])
        else:
            nc.vector.tensor_mul(scr[:], val[:], oh[:])
            nc.vector.tensor_reduce(f_low[:], scr[:], axis=AX.X, op=ALU.add)
            nc.vector.tensor_mul(scr[:], val[:], ohh[:])
            nc.vector.tensor_reduce(f_high[:], scr[:], axis=AX.X, op=ALU.add)
            nc.vector.tensor_mul(scr[:], t128[:], oh[:])
            nc.vector.tensor_reduce(t_low[:], scr[:], axis=AX.X, op=ALU.add)
            nc.vector.tensor_mul(scr[:], t128[:], ohh[:])
            nc.vector.tensor_reduce(t_high[:], scr[:], axis=AX.X, op=ALU.add)

        nc.vector.tensor_mul(t_low[:], t_low[:], st_gc[:, p:p + 1])
        nc.vector.tensor_add(st_dlow[:, p:p + 1], t_low[:], st_dnc[:, p:p + 1])
        nc.vector.tensor_mul(t_high[:], t_high[:], st_gc[:, p:p + 1])
        nc.vector.tensor_add(st_dhigh[:, p:p + 1], t_high[:],
                             st_dnc[:, p:p + 1])
        nc.vector.tensor_copy(st_flow[:, p:p + 1], f_low[:])
        nc.vector.tensor_copy(st_fhigh[:, p:p + 1], f_high[:])

        m1 = work.tile([128, 1], F32, tag="m1")
        nc.vector.tensor_scalar(m1[:], vmin[:], 0.0, None, ALU.is_lt)
        m2 = work.tile([128, 1], F32, tag="m2")
        nc.vector.tensor_scalar(m2[:], f_low[:], 0.0, None, ALU.is_lt)
        nc.vector.tensor_mul(m1[:], m1[:], m2[:])
        nc.vector.tensor_scalar(m2[:], val[:, 0:1], 0.0, None, ALU.is_lt)
        nc.vector.tensor_mul(st_mask[:, p:p + 1], m1[:], m2[:])

    if STOP_AFTER == "scan":
        nc.sync.dma_start(out=d_out.rearrange("(t r) -> r t", r=128),
                          in_=st_dlow[:])
        ctx.close()
        return

    # ================= secant =================
    st_dpred = sing.tile([128, NTILE], F32)

    def secant_dpred():
        den = work.tile([128, NTILE], F32, tag="sden")
        nc.vector.tensor_sub(den[:], st_fhigh[:], st_flow[:])
        dabs = work.tile([128, NTILE], F32, tag="sdabs")
        nc.vector.tensor_scalar(dabs[:].bitcast(U32), den[:].bitcast(U32),
                                0x7FFFFFFF, None, ALU.bitwise_and)
        msk = work.tile([128, NTILE], F32, tag="smsk")
        nc.vector.tensor_scalar(msk[:], dabs[:], 1e-12, None, ALU.is_lt)
        nc.vector.copy_predicated(den[:], msk[:].bitcast(U32), c1em12[:])
        rec = work.tile([128, NTILE], F32, tag="srec")
        nc.vector.reciprocal(rec[:], den[:])
        num = work.tile([128, NTILE], F32, tag="snum")
        nc.vector.tensor_sub(num[:], st_dhigh[:], st_dlow[:])
        nc.vector.tensor_mul(num[:], num[:], st_flow[:])
        nc.vector.tensor_mul(num[:], num[:], rec[:])
        nc.vector.tensor_sub(st_dpred[:], st_dlow[:], num[:])

    secant_dpred()

    dpredd = dram.tile([NSEC + 1, R], F32)   # flattened d_pred per use
    fmd = dram.tile([NSEC + 1, R], F32)      # flattened f / df rows

    def eval_f(i, want_sig=False, tag_final=False):
        nc.sync.dma_start(out=dpredd[i, :].rearrange("(t r) -> r t", r=128),
                          in_=st_dpred[:])
        dfl = rowp.tile([1, R], F32, tag="dflat")
        nc.sync.dma_start(out=dfl[:], in_=dpredd[i:i + 1, :])
        db = bcp.tile([128, R], F32, tag="db")
        nc.gpsimd.partition_broadcast(db[:], dfl[:])
        z4, es = mlp_batch(db[:], None, want_sig, tag_final)
        z4_to_dram(z4, fmd[i:i + 1, :])
        fm = work.tile([128, NTILE], F32, tag="fm")
        nc.sync.dma_start(out=fm[:],
                          in_=fmd[i, :].rearrange("(t r) -> r t", r=128))
        nc.vector.tensor_scalar(fm[:], fm[:], b4col[:], -LOGIT_TAU,
                                ALU.add, ALU.add)
        return fm, db, es

    for it in range(NSEC):
        fm, _, _ = eval_f(it)
        low = work.tile([128, NTILE], F32, tag="lowm")
        nc.vector.tensor_scalar(low[:], fm[:], 0.0, None, ALU.is_lt)
        nc.vector.copy_predicated(st_dlow[:], low[:].bitcast(U32), st_dpred[:])
        nc.vector.copy_predicated(st_flow[:], low[:].bitcast(U32), fm[:])
        hi = work.tile([128, NTILE], F32, tag="him")
        nc.vector.tensor_scalar(hi[:], low[:], -1.0, 1.0, ALU.mult, ALU.add)
        nc.vector.copy_predicated(st_dhigh[:], hi[:].bitcast(U32), st_dpred[:])
        nc.vector.copy_predicated(st_fhigh[:], hi[:].bitcast(U32), fm[:])
        secant_dpred()

    if STOP_AFTER == "secant":
        nc.sync.dma_start(out=d_out.rearrange("(t r) -> r t", r=128),
                          in_=st_dpred[:])
        ctx.close()
        return

    # ================= final newton correction =================
    fval, dbF, es = eval_f(NSEC, want_sig=True, tag_final=True)
    e1, e2, e3 = es

    sigtmp = fin.tile([128, 2 * BP], F32, tag="sigtmp")

    def sig_inplace(e):
        # e <- e / (e + 1)
        nc.vector.tensor_scalar(sigtmp[:], e[:], 1.0, None, ALU.add)
        nc.vector.reciprocal(sigtmp[:], sigtmp[:])
        nc.vector.tensor_mul(e[:], e[:], sigtmp[:])

    sig_inplace(e1)
    sig_inplace(e2)
    sig_inplace(e3)

    dh1 = fin.tile([128, 2 * BP], F32R, tag="dh", name="dh1")
    for c in range(2):
        nc.vector.tensor_mul(dh1[:, c * BP:(c + 1) * BP],
                             e1[:, c * BP:(c + 1) * BP], C[c][:])

    def tangent_layer(dh_in, wk, s_l, tagl):
        dh = fin.tile([128, 2 * BP], F32R, tag="dh", name="dh")
        for c in range(2):
            ps = psA.tile([128, BP], F32, tag="z")
            for k in range(2):
                for hh in range(2):
                    nc.tensor.matmul(
                        ps[:, hh * 512:(hh + 1) * 512], wk[k][c][:],
                        dh_in[:, k * BP + hh * 512:k * BP + (hh + 1) * 512],
                        start=(k == 0), stop=(k == 1))
            nc.vector.tensor_mul(dh[:, c * BP:(c + 1) * BP],
                                 s_l[:, c * BP:(c + 1) * BP], ps[:])
        return dh

    dh2 = tangent_layer(dh1, w2, e2, "2")
    dh3 = tangent_layer(dh2, w3, e3, "3")
    dfp = psZ4.tile([128, BP], F32, tag="z4")
    for k in range(2):
        for hh in range(2):
            nc.tensor.matmul(dfp[0:1, hh * 512:(hh + 1) * 512], w4[k][:],
                             dh3[:, k * BP + hh * 512:k * BP + (hh + 1) * 512],
                             start=(k == 0), stop=(k == 1))
    dfdd_d = dram.tile([1, R], F32)
    row = rowp.tile([1, BP], F32, tag="z4row")
    nc.vector.tensor_copy(row[:], dfp[0:1, :])
    nc.sync.dma_start(out=dfdd_d[:], in_=row[:])
    dfdd = work.tile([128, NTILE], F32, tag="dfdd")
    nc.sync.dma_start(out=dfdd[:],
                      in_=dfdd_d[0, :].rearrange("(t r) -> r t", r=128))

    # clamp: |df|<1e-6 -> sign(df)*1e-6 (df==0 -> +1e-6)
    dneg = work.tile([128, NTILE], F32, tag="dneg")
    nc.vector.tensor_scalar(dneg[:], dfdd[:], 0.0, None, ALU.is_lt)
    dabs = work.tile([128, NTILE], F32, tag="dfabs")
    nc.vector.tensor_scalar(dabs[:].bitcast(U32), dfdd[:].bitcast(U32),
                            0x7FFFFFFF, None, ALU.bitwise_and)
    nc.vector.tensor_scalar(dabs[:], dabs[:], 1e-6, None, ALU.max)
    nc.vector.tensor_scalar(dneg[:], dneg[:], -2.0, 1.0, ALU.mult, ALU.add)
    nc.vector.tensor_mul(dfdd[:], dabs[:], dneg[:])

    rec = work.tile([128, NTILE], F32, tag="recF")
    nc.vector.reciprocal(rec[:], dfdd[:])
    nc.vector.tensor_mul(fval[:], fval[:], rec[:])
    dout = work.tile([128, NTILE], F32, tag="dout")
    nc.vector.tensor_sub(dout[:], st_dpred[:], fval[:])
    nc.vector.tensor_mul(dout[:], dout[:], st_mask[:])

    nc.sync.dma_start(out=d_out.rearrange("(t r) -> r t", r=128), in_=dout[:])

    ctx.close()


# ======================= host-side driver =======================

def _host_constants():
    t = np.linspace(0.0, 1.0, S).astype(np.float32)
    t128 = np.ascontiguousarray(np.broadcast_to(t, (128, S)))
    w = np.arange(S, 0, -1, dtype=np.float32)   # S, S-1, ..., 1
    wcost = np.ascontiguousarray(np.broadcast_to(w, (128, S)))
    ident = np.eye(128, dtype=np.float32)
    return t128, wcost, ident


def kernel(ray0, ray_direction, W1, b1, W2, b2, W3, b3, W4, b4):
    if "prog" not in _nc_cache:
        _nc_cache["prog"] = build_program()
    nc = _nc_cache["prog"]

    t128, wcost, ident = _host_constants()
    r0 = np.ascontiguousarray(np.asarray(ray0).reshape(N, 3), np.float32)
    rd = np.ascontiguousarray(np.asarray(ray_direction).reshape(N, 3),
                              np.float32)
    shared = {
        "W1": np.ascontiguousarray(W1, np.float32),
        "b1": np.ascontiguousarray(b1, np.float32),
        "W2": np.ascontiguousarray(W2, np.float32),
        "b2": np.ascontiguousarray(b2, np.float32),
        "W3": np.ascontiguousarray(W3, np.float32),
        "b3": np.ascontiguousarray(b3, np.float32),
        "W4": np.ascontiguousarray(W4, np.float32),
        "b4": np.ascontiguousarray(b4, np.float32),
        "t128": t128, "wcost": wcost, "ident": ident,
    }
    in_maps = []
    for c in range(NCORES):
        m = dict(shared)
        m["ray0"] = r0[c * R:(c + 1) * R]
        m["rd"] = rd[c * R:(c + 1) * R]
        in_maps.append(m)

    res = run_bass_kernel_spmd(nc, in_maps, core_ids=list(range(NCORES)))
    out = np.concatenate([res.results[c]["out"] for c in range(NCORES)])
    return out.reshape(1, N).astype(np.float32)


# revision 12
# speedup vs baseline: 2.3828x; 2.3828x over previous
"""Trainium2 Bass kernel for nn_DepthModule: ray-marched implicit-surface depth.

kernel(**inputs) takes FULL unsharded inputs (ray0 [1,8192,3], ray_direction
[1,8192,3], MLP weights W1..b4) and returns the FULL output [1,8192] float32.
The N=8192 ray axis is sharded across 8 NeuronCores (data parallel, weights
replicated); each core runs an identical Bass/Tile program on its 1024 rays.

Per-core pipeline (mirrors the jax reference):
  1. cube entry/exit depths per ray (DVE vector math, rays on partitions)
  2. S=128 proposal depths/ray -> MLP occupancy logits for all 128K points
     (fp32r matmuls on PE; softplus(z) = Ln(Exp(z)+1) on the scalar engine,
     biases folded into Exp's pre-activation bias)
  3. first-sign-change scan (one-hot gather via tensor_tensor_reduce)
  4. 8 secant-refinement iterations (1024-point MLP evals)
  5. implicit-gradient Newton correction (forward + JVP tangent pass;
     sigmoid(z) = E/(E+1) recovered from the saved Exp values on DVE)
"""

from contextlib import ExitStack

import numpy as np

import concourse.bacc as bacc
import concourse.bass as bass
import concourse.tile as tile
import concourse.mybir as mybir
from concourse.bass_utils import run_bass_kernel_spmd  # noqa: F401 (fallback)

F32 = mybir.dt.float32
F32R = mybir.dt.float32r
U32 = mybir.dt.uint32
AF = mybir.ActivationFunctionType
ALU = mybir.AluOpType
AX = mybir.AxisListType

NCORES = 8
N = 8192
R = N // NCORES          # rays per core = 1024
S = 128                  # proposal samples per ray
H = 256                  # MLP hidden dim
NSEC = 8                 # secant iterations
NTILE = R // 128         # ray tiles per core = 8
BP = 1024                # points per stage-A batch (8 rays x 128 samples)
NB = R * S // BP         # stage-A batches = 128

TAU = 0.5
LOGIT_TAU = float(np.log(TAU / (1.0 - TAU)))   # 0.0
DEPTH_LO, DEPTH_HI = 0.0, 2.4
PADDING = 0.1
EPS = 1e-6
P_DIST = 0.5 + PADDING / 2.0
BIG = 1e9

_nc_cache = {}
import os
STOP_AFTER = os.environ.get("K_STOP_AFTER", "full")
# InstTensorTensorReduce crashes the exec unit on this toolchain/HW
USE_TTR = os.environ.get("K_USE_TTR", "0") == "1"


def expand_ap(ap, pattern, offset_elems=0):
    """AP over `ap`'s tensor keeping its partition dim, with explicit free
    [stride, count] pairs (strides in elements, outer->inner)."""
    new = [ap.ap[0]] + [[s, c] for (s, c) in pattern]
    return bass.AP(tensor=ap.tensor, offset=ap.offset + offset_elems, ap=new)


def build_program():
    nc = bacc.Bacc("TRN2", target_bir_lowering=False, debug=False,
                   num_devices=NCORES)

    d_ray0 = nc.dram_tensor("ray0", [R, 3], F32, kind="ExternalInput").ap()
    d_rd = nc.dram_tensor("rd", [R, 3], F32, kind="ExternalInput").ap()
    d_W1 = nc.dram_tensor("W1", [3, H], F32, kind="ExternalInput").ap()
    d_b1 = nc.dram_tensor("b1", [H], F32, kind="ExternalInput").ap()
    d_W2 = nc.dram_tensor("W2", [H, H], F32, kind="ExternalInput").ap()
    d_b2 = nc.dram_tensor("b2", [H], F32, kind="ExternalInput").ap()
    d_W3 = nc.dram_tensor("W3", [H, H], F32, kind="ExternalInput").ap()
    d_b3 = nc.dram_tensor("b3", [H], F32, kind="ExternalInput").ap()
    d_W4 = nc.dram_tensor("W4", [H, 1], F32, kind="ExternalInput").ap()
    d_b4 = nc.dram_tensor("b4", [1], F32, kind="ExternalInput").ap()
    d_t128 = nc.dram_tensor("t128", [128, S], F32, kind="ExternalInput").ap()
    d_wcost = nc.dram_tensor("wcost", [128, S], F32, kind="ExternalInput").ap()
    d_ident = nc.dram_tensor("ident", [128, 128], F32, kind="ExternalInput").ap()
    d_out = nc.dram_tensor("out", [R], F32, kind="ExternalOutput").ap()

    with tile.TileContext(nc) as tc:
        _emit(nc, tc, d_ray0, d_rd, d_W1, d_b1, d_W2, d_b2, d_W3, d_b3,
              d_W4, d_b4, d_t128, d_wcost, d_ident, d_out)

    nc.compile()
    return nc


def _emit(nc, tc, d_ray0, d_rd, d_W1, d_b1, d_W2, d_b2, d_W3, d_b3,
          d_W4, d_b4, d_t128, d_wcost, d_ident, d_out):
    ctx = ExitStack()
    sing = ctx.enter_context(tc.tile_pool(name="sing", bufs=1))
    work = ctx.enter_context(tc.tile_pool(name="work", bufs=2))
    z1p = ctx.enter_context(tc.tile_pool(name="z1p", bufs=2))
    ep = ctx.enter_context(tc.tile_pool(name="ep", bufs=2))
    fin = ctx.enter_context(tc.tile_pool(name="fin", bufs=1))
    bcp = ctx.enter_context(tc.tile_pool(name="bcp", bufs=2))
    rowp = ctx.enter_context(tc.tile_pool(name="rowp", bufs=2))
    psA = ctx.enter_context(tc.tile_pool(name="psA", bufs=3, space="PSUM"))
    psZ4 = ctx.enter_context(tc.tile_pool(name="psZ4", bufs=1, space="PSUM"))
    dram = ctx.enter_context(tc.tile_pool(name="dram", bufs=1, space="DRAM"))

    # ================= constants & weights =================
    t128 = sing.tile([128, S], F32)        # t128[p, s] = t[s]
    wcost = sing.tile([128, S], F32)       # sign-scan cost weights
    ident = sing.tile([128, 128], F32)
    nc.sync.dma_start(out=t128[:], in_=d_t128)
    nc.sync.dma_start(out=wcost[:], in_=d_wcost)
    nc.sync.dma_start(out=ident[:], in_=d_ident)

    w2 = [[sing.tile([128, 128], F32R, tag=f"w2_{k}_{c}", name=f"w2_{k}_{c}")
           for c in range(2)] for k in range(2)]
    w3 = [[sing.tile([128, 128], F32R, tag=f"w3_{k}_{c}", name=f"w3_{k}_{c}")
           for c in range(2)] for k in range(2)]
    w4 = [sing.tile([128, 1], F32R, tag=f"w4_{k}", name=f"w4_{k}")
          for k in range(2)]
    w1 = [sing.tile([3, 128], F32R, tag=f"w1_{c}", name=f"w1_{c}")
          for c in range(2)]
    for k in range(2):
        for c in range(2):
            nc.sync.dma_start(out=w2[k][c][:],
                              in_=d_W2[k * 128:(k + 1) * 128,
                                       c * 128:(c + 1) * 128].bitcast(F32R))
            nc.sync.dma_start(out=w3[k][c][:],
                              in_=d_W3[k * 128:(k + 1) * 128,
                                       c * 128:(c + 1) * 128].bitcast(F32R))
        nc.sync.dma_start(out=w4[k][:],
                          in_=d_W4[k * 128:(k + 1) * 128, :].bitcast(F32R))
    for c in range(2):
        nc.sync.dma_start(out=w1[c][:],
                          in_=d_W1[:, c * 128:(c + 1) * 128].bitcast(F32R))

    b1c = [sing.tile([128, 1], F32, tag=f"b1_{c}", name=f"b1_{c}") for c in range(2)]
    b2c = [sing.tile([128, 1], F32, tag=f"b2_{c}", name=f"b2_{c}") for c in range(2)]
    b3c = [sing.tile([128, 1], F32, tag=f"b3_{c}", name=f"b3_{c}") for c in range(2)]
    for c in range(2):
        nc.sync.dma_start(out=b1c[c][:], in_=d_b1[c * 128:(c + 1) * 128]
                          .rearrange("(h o) -> h o", o=1))
        nc.sync.dma_start(out=b2c[c][:], in_=d_b2[c * 128:(c + 1) * 128]
                          .rearrange("(h o) -> h o", o=1))
        nc.sync.dma_start(out=b3c[c][:], in_=d_b3[c * 128:(c + 1) * 128]
                          .rearrange("(h o) -> h o", o=1))
    b4col = sing.tile([128, 1], F32)
    nc.sync.dma_start(out=b4col[:],
                      in_=bass.AP(tensor=d_b4.tensor, offset=d_b4.offset,
                                  ap=[[0, 128], [1, 1]]))

    r0T = sing.tile([3, R], F32R)
    rdT = sing.tile([3, R], F32R)
    nc.sync.dma_start(out=r0T[:], in_=d_ray0.rearrange("n c -> c n").bitcast(F32R))
    nc.sync.dma_start(out=rdT[:], in_=d_rd.rearrange("n c -> c n").bitcast(F32R))

    # ================= A/C layer-1 folds =================
    # A[c][f, ray] = (W1^T ray0^T)[f, ray] + b1[f];  C[c] = W1^T rd^T
    A = [sing.tile([128, R], F32, tag=f"A_{c}", name=f"A_{c}") for c in range(2)]
    C = [sing.tile([128, R], F32, tag=f"C_{c}", name=f"C_{c}") for c in range(2)]
    for c in range(2):
        for h in range(2):
            ps = psA.tile([128, BP], F32, tag="z")
            nc.tensor.matmul(ps[:, 0:512], w1[c][:],
                             r0T[:, h * 512:(h + 1) * 512],
                             start=True, stop=True)
            nc.scalar.activation(A[c][:, h * 512:(h + 1) * 512], ps[:, 0:512],
                                 AF.Identity, bias=b1c[c][:])
            ps2 = psA.tile([128, BP], F32, tag="z")
            nc.tensor.matmul(ps2[:, 0:512], w1[c][:],
                             rdT[:, h * 512:(h + 1) * 512],
                             start=True, stop=True)
            nc.scalar.activation(C[c][:, h * 512:(h + 1) * 512], ps2[:, 0:512],
                                 AF.Identity)

    # ================= cube intersection =================
    statsd = dram.tile([NTILE, 2, 128], F32)   # [tile][dn_sel; g_sel][ray]
    bigt = sing.tile([128, 6], F32)
    nc.vector.memset(bigt[:], BIG)
    c1e9 = sing.tile([128, 3], F32)
    nc.vector.memset(c1e9[:], 1e-9)
    c1em12 = sing.tile([128, NTILE], F32)
    nc.vector.memset(c1em12[:], 1e-12)

    for p in range(NTILE):
        r0t = work.tile([128, 3], F32, tag="r0t")
        rdt = work.tile([128, 3], F32, tag="rdt")
        nc.sync.dma_start(out=r0t[:], in_=d_ray0[p * 128:(p + 1) * 128, :])
        nc.sync.dma_start(out=rdt[:], in_=d_rd[p * 128:(p + 1) * 128, :])

        # den = where(|rd| < 1e-9, 1e-9, rd)
        den = work.tile([128, 3], F32, tag="den")
        nc.vector.tensor_scalar(den[:].bitcast(U32), rdt[:].bitcast(U32),
                                0x7FFFFFFF, None, ALU.bitwise_and)
        msmall = work.tile([128, 3], F32, tag="msmall")
        nc.vector.tensor_scalar(msmall[:], den[:], 1e-9, None, ALU.is_lt)
        nc.vector.tensor_copy(den[:], rdt[:])
        nc.vector.copy_predicated(den[:], msmall[:].bitcast(U32), c1e9[:])
        inv = work.tile([128, 3], F32, tag="inv")
        nc.vector.reciprocal(inv[:], den[:])

        # d6 = (plane - ray0) * inv, planes (+p,+p,+p,-p,-p,-p)
        d6 = work.tile([128, 6], F32, tag="d6")
        nom = work.tile([128, 3], F32, tag="nom")
        nc.vector.tensor_scalar(nom[:], r0t[:], -1.0, P_DIST, ALU.mult, ALU.add)
        nc.vector.tensor_mul(d6[:, 0:3], nom[:], inv[:])
        nc.vector.tensor_scalar(nom[:], r0t[:], -1.0, -P_DIST, ALU.mult, ALU.add)
        nc.vector.tensor_mul(d6[:, 3:6], nom[:], inv[:])

        # p_int[r, k, j] = ray0[r, j] + d6[r, k] * rd[r, j]
        pi = work.tile([128, 18], F32, tag="pi")
        nc.vector.tensor_tensor(pi[:], expand_ap(d6[:], [(1, 6), (0, 3)]),
                                expand_ap(rdt[:], [(0, 6), (1, 3)]), ALU.mult)
        nc.vector.tensor_tensor(pi[:], pi[:],
                                expand_ap(r0t[:], [(0, 6), (1, 3)]), ALU.add)
        nc.vector.tensor_scalar(pi[:].bitcast(U32), pi[:].bitcast(U32),
                                0x7FFFFFFF, None, ALU.bitwise_and)
        nc.vector.tensor_scalar(pi[:], pi[:], P_DIST + EPS, None, ALU.is_le)
        in6 = work.tile([128, 6], F32, tag="in6")
        nc.vector.tensor_reduce(in6[:], pi[:].rearrange("p (k j) -> p k j", j=3),
                                axis=AX.X, op=ALU.min)
        cnt = work.tile([128, 1], F32, tag="cnt")
        nc.vector.tensor_reduce(cnt[:], in6[:], axis=AX.X, op=ALU.add)
        mcube = work.tile([128, 1], F32, tag="mcube")
        nc.vector.tensor_scalar(mcube[:], cnt[:], 2.0, None, ALU.is_equal)

        # dn = min inside |d|, df = max inside |d|
        nc.vector.tensor_scalar(d6[:].bitcast(U32), d6[:].bitcast(U32),
                                0x7FFFFFFF, None, ALU.bitwise_and)
        lo6 = work.tile([128, 6], F32, tag="lo6")
        nc.vector.tensor_copy(lo6[:], bigt[:])
        nc.vector.copy_predicated(lo6[:], in6[:].bitcast(U32), d6[:])
        dn_r = work.tile([128, 1], F32, tag="dn_r")
        nc.vector.tensor_reduce(dn_r[:], lo6[:], axis=AX.X, op=ALU.min)
        hi6 = work.tile([128, 6], F32, tag="hi6")
        nc.vector.tensor_mul(hi6[:], in6[:], d6[:])
        df_r = work.tile([128, 1], F32, tag="df_r")
        nc.vector.tensor_reduce(df_r[:], hi6[:], axis=AX.X, op=ALU.max)

        # st2 = [mcube*dn, mcube*(df-dn-2.4)+2.4]
        st2 = work.tile([128, 2], F32, tag="st2")
        g_r = work.tile([128, 1], F32, tag="g_r")
        nc.vector.tensor_sub(g_r[:], df_r[:], dn_r[:])
        nc.vector.tensor_mul(st2[:, 0:1], mcube[:], dn_r[:])
        nc.vector.tensor_scalar(g_r[:], g_r[:], 1.0, -(DEPTH_HI - DEPTH_LO),
                                ALU.mult, ALU.add)
        nc.vector.tensor_mul(g_r[:], g_r[:], mcube[:])
        nc.vector.tensor_scalar(st2[:, 1:2], g_r[:], 1.0, DEPTH_HI - DEPTH_LO,
                                ALU.mult, ALU.add)

        pst = psZ4.tile([128, BP], F32, tag="z4")
        nc.tensor.transpose(pst[0:2, 0:128], st2[:], ident[:])
        strow = work.tile([2, 128], F32, tag="strow")
        nc.vector.tensor_copy(strow[:], pst[0:2, 0:128])
        nc.sync.dma_start(out=statsd[p, :, :], in_=strow[:])

    # (tile,k)-partition layouts for the proposal-depth construction
    dn128 = sing.tile([128, NTILE], F32)
    g128 = sing.tile([128, NTILE], F32)
    for p in range(NTILE):
        nc.sync.dma_start(out=dn128[p * 16:(p + 1) * 16, :],
                          in_=statsd[p, 0, :].rearrange("(k r) -> k r", r=8))
        nc.sync.dma_start(out=g128[p * 16:(p + 1) * 16, :],
                          in_=statsd[p, 1, :].rearrange("(k r) -> k r", r=8))
    # (ray,tile) layouts for the scan / secant math
    st_dnc = sing.tile([128, NTILE], F32)
    st_gc = sing.tile([128, NTILE], F32)
    nc.sync.dma_start(out=st_dnc[:], in_=statsd[:, 0, :].rearrange("p r -> r p"))
    nc.sync.dma_start(out=st_gc[:], in_=statsd[:, 1, :].rearrange("p r -> r p"))

    if STOP_AFTER == "cube":
        nc.sync.dma_start(out=d_out.rearrange("(t r) -> r t", r=128),
                          in_=st_dnc[:])
        ctx.close()
        return

    # ================= proposal depths D8b =================
    # D8b[(p,k), (r8, s)] = dn128[(p,k), r8] + t[s] * g128[(p,k), r8]
    D8b = sing.tile([128, BP], F32)
    nc.vector.tensor_tensor(D8b[:], expand_ap(t128[:], [(0, 8), (1, S)]),
                            expand_ap(g128[:], [(1, 8), (0, S)]), ALU.mult)
    nc.vector.tensor_tensor(D8b[:], D8b[:],
                            expand_ap(dn128[:], [(1, 8), (0, S)]), ALU.add)

    vald = dram.tile([NB, BP], F32)     # stage-A logits, flat batch rows
    d8d = dram.tile([NB, BP], F32)      # proposal depths, flat batch rows
    nc.sync.dma_start(out=d8d[:], in_=D8b[:])

    # ================= MLP forward for one 1024-point batch ==============
    def mlp_batch(db_ap, per_ray, want_sig, tag_final=False):
        """db_ap: [128, 1024] depth broadcast. per_ray=(p0,k0) selects
        stage-A per-ray scalars; None means point index == ray index.
        Returns (z4 psum tile, e_list or None)."""
        z1 = z1p.tile([128, 2 * BP], F32, tag="z1")
        if per_ray is not None:
            p0, k0 = per_ray
            g0 = p0 * 128 + k0 * 8
            for c in range(2):
                for j in range(8):
                    nc.vector.tensor_scalar(
                        z1[:, c * BP + j * 128:c * BP + (j + 1) * 128],
                        db_ap[:, j * 128:(j + 1) * 128],
                        C[c][:, g0 + j:g0 + j + 1],
                        A[c][:, g0 + j:g0 + j + 1],
                        ALU.mult, ALU.add)
        else:
            for c in range(2):
                nc.vector.tensor_mul(z1[:, c * BP:(c + 1) * BP], db_ap, C[c][:])
                nc.vector.tensor_add(z1[:, c * BP:(c + 1) * BP],
                                     z1[:, c * BP:(c + 1) * BP], A[c][:])

        if want_sig:
            e1 = fin.tile([128, 2 * BP], F32, tag="e1f", name="e1f")
        else:
            e1 = ep.tile([128, 2 * BP], F32, tag="e", name="e1")
        nc.scalar.activation(e1[:], z1[:], AF.Exp)
        if want_sig:
            h1 = fin.tile([128, 2 * BP], F32R, tag="h1f", name="h1f")
        else:
            h1 = ep.tile([128, 2 * BP], F32R, tag="h", name="h1")
        nc.scalar.activation(h1[:], e1[:], AF.Ln, bias=1.0)
        es = [e1]

        def layer(h_in, wk, bk, tagl):
            if want_sig:
                e = fin.tile([128, 2 * BP], F32, tag="e" + tagl + "f",
                             name="e" + tagl + "f")
            else:
                e = ep.tile([128, 2 * BP], F32, tag="e", name="e" + tagl)
            for c in range(2):
                ps = psA.tile([128, BP], F32, tag="z")
                for k in range(2):
                    for hh in range(2):
                        nc.tensor.matmul(
                            ps[:, hh * 512:(hh + 1) * 512], wk[k][c][:],
                            h_in[:, k * BP + hh * 512:k * BP + (hh + 1) * 512],
                            start=(k == 0), stop=(k == 1))
                nc.scalar.activation(e[:, c * BP:(c + 1) * BP], ps[:],
                                     AF.Exp, bias=bk[c][:])
            if want_sig:
                h = fin.tile([128, 2 * BP], F32R, tag="h" + tagl + "f",
                             name="h" + tagl + "f")
            else:
                h = ep.tile([128, 2 * BP], F32R, tag="h", name="h" + tagl)
            nc.scalar.activation(h[:], e[:], AF.Ln, bias=1.0)
            es.append(e)
            return h

        h2 = layer(h1, w2, b2c, "2")
        h3 = layer(h2, w3, b3c, "3")
        z4 = psZ4.tile([128, BP], F32, tag="z4")
        for k in range(2):
            for hh in range(2):
                nc.tensor.matmul(z4[0:1, hh * 512:(hh + 1) * 512], w4[k][:],
                                 h3[:, k * BP + hh * 512:k * BP + (hh + 1) * 512],
                                 start=(k == 0), stop=(k == 1))
        return z4, (es if want_sig else None)

    def z4_to_dram(z4, dst_row_ap):
        row = rowp.tile([1, BP], F32, tag="z4row")
        nc.vector.tensor_copy(row[:], z4[0:1, :])
        nc.sync.dma_start(out=dst_row_ap, in_=row[:])

    # ================= stage A =================
    for b in range(NB):
        p0, k0 = b // 16, b % 16
        drow = rowp.tile([1, BP], F32, tag="drow")
        nc.sync.dma_start(out=drow[:], in_=d8d[b:b + 1, :])
        db = bcp.tile([128, BP], F32, tag="db")
        nc.gpsimd.partition_broadcast(db[:], drow[:])
        z4, _ = mlp_batch(db[:], (p0, k0), False)
        z4_to_dram(z4, vald[b:b + 1, :])

    if STOP_AFTER == "stageA":
        tmpo = work.tile([128, NTILE], F32, tag="tmpo")
        for p in range(NTILE):
            nc.sync.dma_start(
                out=tmpo[:, p:p + 1],
                in_=vald[p * 16:(p + 1) * 16, :].rearrange(
                    "k (r s) -> (k r) s", s=S)[:, 0:1])
        nc.sync.dma_start(out=d_out.rearrange("(t r) -> r t", r=128),
                          in_=tmpo[:])
        ctx.close()
        return

    # ================= scan =================
    st_dlow = sing.tile([128, NTILE], F32)
    st_flow = sing.tile([128, NTILE], F32)
    st_dhigh = sing.tile([128, NTILE], F32)
    st_fhigh = sing.tile([128, NTILE], F32)
    st_mask = sing.tile([128, NTILE], F32)

    for p in range(NTILE):
        val = work.tile([128, S], F32, tag="val")
        nc.sync.dma_start(
            out=val[:],
            in_=vald[p * 16:(p + 1) * 16, :].rearrange(
                "k (r s) -> (k r) s", s=S))
        nc.vector.tensor_scalar(val[:], val[:], b4col[:], -LOGIT_TAU,
                                ALU.add, ALU.add)

        prod = work.tile([128, S], F32, tag="prod")
        nc.vector.tensor_tensor(prod[:, 0:S - 1], val[:, 0:S - 1],
                                val[:, 1:S], ALU.mult)
        sgn = work.tile([128, S], F32, tag="sgn")
        nc.scalar.activation(sgn[:, 0:S - 1], prod[:, 0:S - 1], AF.Sign)
        nc.vector.memset(sgn[:, S - 1:S], 1.0)
        cost = work.tile([128, S], F32, tag="cost")
        nc.vector.tensor_tensor(cost[:], sgn[:], wcost[:], ALU.mult)
        vmin = work.tile([128, 1], F32, tag="vmin")
        nc.vector.tensor_reduce(vmin[:], cost[:], axis=AX.X, op=ALU.min)
        oh = work.tile([128, S], F32, tag="oh")
        nc.vector.tensor_scalar(oh[:], cost[:], vmin[:], None, ALU.is_equal)
        ohh = work.tile([128, S], F32, tag="ohh")
        nc.vector.memset(ohh[:, 0:1], 0.0)
        nc.vector.tensor_copy(ohh[:, 1:S], oh[:, 0:S - 1])
        nc.vector.tensor_add(ohh[:, S - 1:S], ohh[:, S - 1:S], oh[:, S - 1:S])

        scr = work.tile([128, S], F32, tag="scr")
        f_low = work.tile([128, 1], F32, tag="f_low")
        f_high = work.tile([128, 1], F32, tag="f_high")
        t_low = work.tile([128, 1], F32, tag="t_low")
        t_high = work.tile([128, 1], F32, tag="t_high")
        if USE_TTR:
            nc.vector.tensor_tensor_reduce(scr[:], val[:], oh[:], 1.0, 0.0,
                                           ALU.mult, ALU.add, f_low[:])
            nc.vector.tensor_tensor_reduce(scr[:], val[:], ohh[:], 1.0, 0.0,
                                           ALU.mult, ALU.add, f_high[:])
            nc.vector.tensor_tensor_reduce(scr[:], t128[:], oh[:], 1.0, 0.0,
                                           ALU.mult, ALU.add, t_low[:])
            nc.vector.tensor_tensor_reduce(scr[:], t128[:], ohh[:], 1.0, 0.0,
                                           ALU.mult, ALU.add, t_high[:])
        else:
            nc.vector.tensor_mul(scr[:], val[:], oh[:])
            nc.vector.tensor_reduce(f_low[:], scr[:], axis=AX.X, op=ALU.add)
            nc.vector.tensor_mul(scr[:], val[:], ohh[:])
            nc.vector.tensor_reduce(f_high[:], scr[:], axis=AX.X, op=ALU.add)
            nc.vector.tensor_mul(scr[:], t128[:], oh[:])
            nc.vector.tensor_reduce(t_low[:], scr[:], axis=AX.X, op=ALU.add)
            nc.vector.tensor_mul(scr[:], t128[:], ohh[:])
            nc.vector.tensor_reduce(t_high[:], scr[:], axis=AX.X, op=ALU.add)

        nc.vector.tensor_mul(t_low[:], t_low[:], st_gc[:, p:p + 1])
        nc.vector.tensor_add(st_dlow[:, p:p + 1], t_low[:], st_dnc[:, p:p + 1])
        nc.vector.tensor_mul(t_high[:], t_high[:], st_gc[:, p:p + 1])
        nc.vector.tensor_add(st_dhigh[:, p:p + 1], t_high[:],
                             st_dnc[:, p:p + 1])
        nc.vector.tensor_copy(st_flow[:, p:p + 1], f_low[:])
        nc.vector.tensor_copy(st_fhigh[:, p:p + 1], f_high[:])

        m1 = work.tile([128, 1], F32, tag="m1")
        nc.vector.tensor_scalar(m1[:], vmin[:], 0.0, None, ALU.is_lt)
        m2 = work.tile([128, 1], F32, tag="m2")
        nc.vector.tensor_scalar(m2[:], f_low[:], 0.0, None, ALU.is_lt)
        nc.vector.tensor_mul(m1[:], m1[:], m2[:])
        nc.vector.tensor_scalar(m2[:], val[:, 0:1], 0.0, None, ALU.is_lt)
        nc.vector.tensor_mul(st_mask[:, p:p + 1], m1[:], m2[:])

    if STOP_AFTER == "scan":
        nc.sync.dma_start(out=d_out.rearrange("(t r) -> r t", r=128),
                          in_=st_dlow[:])
        ctx.close()
        return

    # ================= secant =================
    st_dpred = sing.tile([128, NTILE], F32)

    def secant_dpred():
        den = work.tile([128, NTILE], F32, tag="sden")
        nc.vector.tensor_sub(den[:], st_fhigh[:], st_flow[:])
        dabs = work.tile([128, NTILE], F32, tag="sdabs")
        nc.vector.tensor_scalar(dabs[:].bitcast(U32), den[:].bitcast(U32),
                                0x7FFFFFFF, None, ALU.bitwise_and)
        msk = work.tile([128, NTILE], F32, tag="smsk")
        nc.vector.tensor_scalar(msk[:], dabs[:], 1e-12, None, ALU.is_lt)
        nc.vector.copy_predicated(den[:], msk[:].bitcast(U32), c1em12[:])
        rec = work.tile([128, NTILE], F32, tag="srec")
        nc.vector.reciprocal(rec[:], den[:])
        num = work.tile([128, NTILE], F32, tag="snum")
        nc.vector.tensor_sub(num[:], st_dhigh[:], st_dlow[:])
        nc.vector.tensor_mul(num[:], num[:], st_flow[:])
        nc.vector.tensor_mul(num[:], num[:], rec[:])
        nc.vector.tensor_sub(st_dpred[:], st_dlow[:], num[:])

    secant_dpred()

    dpredd = dram.tile([NSEC + 1, R], F32)   # flattened d_pred per use
    fmd = dram.tile([NSEC + 1, R], F32)      # flattened f / df rows

    def eval_f(i, want_sig=False, tag_final=False):
        nc.sync.dma_start(out=dpredd[i, :].rearrange("(t r) -> r t", r=128),
                          in_=st_dpred[:])
        dfl = rowp.tile([1, R], F32, tag="dflat")
        nc.sync.dma_start(out=dfl[:], in_=dpredd[i:i + 1, :])
        db = bcp.tile([128, R], F32, tag="db")
        nc.gpsimd.partition_broadcast(db[:], dfl[:])
        z4, es = mlp_batch(db[:], None, want_sig, tag_final)
        z4_to_dram(z4, fmd[i:i + 1, :])
        fm = work.tile([128, NTILE], F32, tag="fm")
        nc.sync.dma_start(out=fm[:],
                          in_=fmd[i, :].rearrange("(t r) -> r t", r=128))
        nc.vector.tensor_scalar(fm[:], fm[:], b4col[:], -LOGIT_TAU,
                                ALU.add, ALU.add)
        return fm, db, es

    for it in range(NSEC):
        fm, _, _ = eval_f(it)
        low = work.tile([128, NTILE], F32, tag="lowm")
        nc.vector.tensor_scalar(low[:], fm[:], 0.0, None, ALU.is_lt)
        nc.vector.copy_predicated(st_dlow[:], low[:].bitcast(U32), st_dpred[:])
        nc.vector.copy_predicated(st_flow[:], low[:].bitcast(U32), fm[:])
        hi = work.tile([128, NTILE], F32, tag="him")
        nc.vector.tensor_scalar(hi[:], low[:], -1.0, 1.0, ALU.mult, ALU.add)
        nc.vector.copy_predicated(st_dhigh[:], hi[:].bitcast(U32), st_dpred[:])
        nc.vector.copy_predicated(st_fhigh[:], hi[:].bitcast(U32), fm[:])
        secant_dpred()

    if STOP_AFTER == "secant":
        nc.sync.dma_start(out=d_out.rearrange("(t r) -> r t", r=128),
                          in_=st_dpred[:])
        ctx.close()
        return

    # ================= final newton correction =================
    fval, dbF, es = eval_f(NSEC, want_sig=True, tag_final=True)
    e1, e2, e3 = es

    sigtmp = fin.tile([128, 2 * BP], F32, tag="sigtmp")

    def sig_inplace(e):
        # e <- e / (e + 1)
        nc.vector.tensor_scalar(sigtmp[:], e[:], 1.0, None, ALU.add)
        nc.vector.reciprocal(sigtmp[:], sigtmp[:])
        nc.vector.tensor_mul(e[:], e[:], sigtmp[:])

    sig_inplace(e1)
    sig_inplace(e2)
    sig_inplace(e3)

    dh1 = fin.tile([128, 2 * BP], F32R, tag="dh", name="dh1")
    for c in range(2):
        nc.vector.tensor_mul(dh1[:, c * BP:(c + 1) * BP],
                             e1[:, c * BP:(c + 1) * BP], C[c][:])

    def tangent_layer(dh_in, wk, s_l, tagl):
        dh = fin.tile([128, 2 * BP], F32R, tag="dh", name="dh")
        for c in range(2):
            ps = psA.tile([128, BP], F32, tag="z")
            for k in range(2):
                for hh in range(2):
                    nc.tensor.matmul(
                        ps[:, hh * 512:(hh + 1) * 512], wk[k][c][:],
                        dh_in[:, k * BP + hh * 512:k * BP + (hh + 1) * 512],
                        start=(k == 0), stop=(k == 1))
            nc.vector.tensor_mul(dh[:, c * BP:(c + 1) * BP],
                                 s_l[:, c * BP:(c + 1) * BP], ps[:])
        return dh

    dh2 = tangent_layer(dh1, w2, e2, "2")
    dh3 = tangent_layer(dh2, w3, e3, "3")
    dfp = psZ4.tile([128, BP], F32, tag="z4")
    for k in range(2):
        for hh in range(2):
            nc.tensor.matmul(dfp[0:1, hh * 512:(hh + 1) * 512], w4[k][:],
                             dh3[:, k * BP + hh * 512:k * BP + (hh + 1) * 512],
                             start=(k == 0), stop=(k == 1))
    dfdd_d = dram.tile([1, R], F32)
    row = rowp.tile([1, BP], F32, tag="z4row")
    nc.vector.tensor_copy(row[:], dfp[0:1, :])
    nc.sync.dma_start(out=dfdd_d[:], in_=row[:])
    dfdd = work.tile([128, NTILE], F32, tag="dfdd")
    nc.sync.dma_start(out=dfdd[:],
                      in_=dfdd_d[0, :].rearrange("(t r) -> r t", r=128))

    # clamp: |df|<1e-6 -> sign(df)*1e-6 (df==0 -> +1e-6)
    dneg = work.tile([128, NTILE], F32, tag="dneg")
    nc.vector.tensor_scalar(dneg[:], dfdd[:], 0.0, None, ALU.is_lt)
    dabs = work.tile([128, NTILE], F32, tag="dfabs")
    nc.vector.tensor_scalar(dabs[:].bitcast(U32), dfdd[:].bitcast(U32),
                            0x7FFFFFFF, None, ALU.bitwise_and)
    nc.vector.tensor_scalar(dabs[:], dabs[:], 1e-6, None, ALU.max)
    nc.vector.tensor_scalar(dneg[:], dneg[:], -2.0, 1.0, ALU.mult, ALU.add)
    nc.vector.tensor_mul(dfdd[:], dabs[:], dneg[:])

    rec = work.tile([128, NTILE], F32, tag="recF")
    nc.vector.reciprocal(rec[:], dfdd[:])
    nc.vector.tensor_mul(fval[:], fval[:], rec[:])
    dout = work.tile([128, NTILE], F32, tag="dout")
    nc.vector.tensor_sub(dout[:], st_dpred[:], fval[:])
    nc.vector.tensor_mul(dout[:], dout[:], st_mask[:])

    nc.sync.dma_start(out=d_out.rearrange("(t r) -> r t", r=128), in_=dout[:])

    ctx.close()


# ======================= host-side driver =======================

def _host_constants():
    t = np.linspace(0.0, 1.0, S).astype(np.float32)
    t128 = np.ascontiguousarray(np.broadcast_to(t, (128, S)))
    w = np.arange(S, 0, -1, dtype=np.float32)   # S, S-1, ..., 1
    wcost = np.ascontiguousarray(np.broadcast_to(w, (128, S)))
    ident = np.eye(128, dtype=np.float32)
    return t128, wcost, ident


def _make_runner(nc):
    """Persistent 8-core PJRT runner (mirrors bass2jax.run_bass_via_pjrt's
    multi-core path, but keeps the compiled shard_map callable across calls)."""
    import jax
    from jax.sharding import Mesh, PartitionSpec
    from jax.experimental.shard_map import shard_map
    from concourse import bass2jax
    from concourse import mybir as _mb

    bass2jax.install_neuronx_cc_hook()
    partition_name = (nc.partition_id_tensor.name
                      if nc.partition_id_tensor else None)
    in_names, out_names, out_avals, zero_shapes = [], [], [], []
    for alloc in nc.m.functions[0].allocations:
        if not isinstance(alloc, _mb.MemoryLocationSet):
            continue
        name = alloc.memorylocations[0].name
        if alloc.kind == "ExternalInput":
            if name != partition_name:
                in_names.append(name)
        elif alloc.kind == "ExternalOutput":
            out_names.append(name)
            shape = tuple(alloc.tensor_shape)
            dtype = _mb.dt.np(alloc.dtype)
            out_avals.append(jax.core.ShapedArray(shape, dtype))
            zero_shapes.append((shape, dtype))
    n_params = len(in_names)
    n_outs = len(out_avals)
    all_names = list(in_names) + list(out_names)
    if partition_name is not None:
        all_names.append(partition_name)
    donate = tuple(range(n_params, n_params + n_outs))

    def _body(*args):
        operands = list(args)
        if partition_name is not None:
            operands.append(bass2jax.partition_id_tensor())
        outs = bass2jax._bass_exec_p.bind(
            *operands,
            out_avals=tuple(out_avals),
            in_names=tuple(all_names),
            out_names=tuple(out_names),
            lowering_input_output_aliases=(),
            sim_require_finite=True,
            sim_require_nnan=True,
            nc=nc,
        )
        return tuple(outs)

    devices = jax.devices()[:NCORES]
    mesh = Mesh(np.asarray(devices), ("core",))
    in_specs = (PartitionSpec("core"),) * (n_params + n_outs)
    out_specs = (PartitionSpec("core"),) * n_outs
    sharded = jax.jit(
        shard_map(_body, mesh=mesh, in_specs=in_specs, out_specs=out_specs,
                  check_rep=False),
        donate_argnums=donate, keep_unused=True)

    def run(in_maps):
        concat_in = [
            np.concatenate([np.asarray(in_maps[c][nm]) for c in range(NCORES)],
                           axis=0)
            for nm in in_names
        ]
        concat_zeros = [np.zeros((NCORES * sh[0], *sh[1:]), dt)
                        for (sh, dt) in zero_shapes]
        outs = jax.block_until_ready(sharded(*concat_in, *concat_zeros))
        return [
            {nm: np.asarray(outs[i]).reshape(NCORES, *out_avals[i].shape)[c]
             for i, nm in enumerate(out_names)}
            for c in range(NCORES)
        ]

    return run


def kernel(ray0, ray_direction, W1, b1, W2, b2, W3, b3, W4, b4):
    if "prog" not in _nc_cache:
        _nc_cache["prog"] = build_program()
        _nc_cache["runner"] = _make_runner(_nc_cache["prog"])
    nc = _nc_cache["prog"]

    t128, wcost, ident = _host_constants()
    r0 = np.ascontiguousarray(np.asarray(ray0).reshape(N, 3), np.float32)
    rd = np.ascontiguousarray(np.asarray(ray_direction).reshape(N, 3),
                              np.float32)
    shared = {
        "W1": np.ascontiguousarray(W1, np.float32),
        "b1": np.ascontiguousarray(b1, np.float32),
        "W2": np.ascontiguousarray(W2, np.float32),
        "b2": np.ascontiguousarray(b2, np.float32),
        "W3": np.ascontiguousarray(W3, np.float32),
        "b3": np.ascontiguousarray(b3, np.float32),
        "W4": np.ascontiguousarray(W4, np.float32),
        "b4": np.ascontiguousarray(b4, np.float32),
        "t128": t128, "wcost": wcost, "ident": ident,
    }
    in_maps = []
    for c in range(NCORES):
        m = dict(shared)
        m["ray0"] = r0[c * R:(c + 1) * R]
        m["rd"] = rd[c * R:(c + 1) * R]
        in_maps.append(m)

    results = _nc_cache["runner"](in_maps)
    out = np.concatenate([results[c]["out"] for c in range(NCORES)])
    return out.reshape(1, N).astype(np.float32)


# revision 21
# speedup vs baseline: 2.7817x; 1.1674x over previous
"""Trainium2 Bass kernel for nn_DepthModule: ray-marched implicit-surface depth.

kernel(**inputs) takes FULL unsharded inputs (ray0 [1,8192,3], ray_direction
[1,8192,3], MLP weights W1..b4) and returns the FULL output [1,8192] float32.
The N=8192 ray axis is sharded across 8 NeuronCores (data parallel, weights
replicated); each core runs an identical Bass/Tile program on its 1024 rays.

Per-core pipeline (mirrors the jax reference):
  1. cube entry/exit depths per ray (DVE vector math, rays on partitions)
  2. S=128 proposal depths/ray -> MLP occupancy logits for all 128K points
     (fp32r matmuls on PE; softplus(z) = Ln(Exp(z)+1) on the scalar engine,
     biases folded into Exp's pre-activation bias)
  3. first-sign-change scan (one-hot gather via tensor_tensor_reduce)
  4. 8 secant-refinement iterations (1024-point MLP evals)
  5. implicit-gradient Newton correction (forward + JVP tangent pass;
     sigmoid(z) = E/(E+1) recovered from the saved Exp values on DVE)
"""

from contextlib import ExitStack

import numpy as np

import concourse.bacc as bacc
import concourse.bass as bass
import concourse.tile as tile
import concourse.mybir as mybir
from concourse.bass_utils import run_bass_kernel_spmd  # noqa: F401 (fallback)

F32 = mybir.dt.float32
F32R = mybir.dt.float32r
U32 = mybir.dt.uint32
AF = mybir.ActivationFunctionType
ALU = mybir.AluOpType
AX = mybir.AxisListType

NCORES = 8
N = 8192
R = N // NCORES          # rays per core = 1024
S = 128                  # proposal samples per ray
H = 256                  # MLP hidden dim
NSEC = 8                 # secant iterations
NTILE = R // 128         # ray tiles per core = 8
BP = 1024                # points per stage-A batch (8 rays x 128 samples)
NB = R * S // BP         # stage-A batches = 128

TAU = 0.5
LOGIT_TAU = float(np.log(TAU / (1.0 - TAU)))   # 0.0
DEPTH_LO, DEPTH_HI = 0.0, 2.4
PADDING = 0.1
EPS = 1e-6
P_DIST = 0.5 + PADDING / 2.0
BIG = 1e9

_nc_cache = {}

# The act-table placement pass picks tables greedily per function; with Exp
# and Ln living in several tables it alternates exp_and_others/natural_log
# loads (~570 loads, 730us of ACT time). Restrict the bass-side view so
# natural_log_exp_and_others is the only table with Exp/Ln -> one load total.
# (Indices into act_info.json are preserved; walrus adopts the pre-placed
# load ids.)
import concourse.hw_specs as _hw_specs
import concourse.bacc as _bacc_mod

_orig_get_tables = _hw_specs.get_activation_tables


def _patched_get_tables(arch):
    t = _orig_get_tables(arch)
    keep = "natural_log_exp_and_others"
    for name, fns in t.items():
        if name == keep:
            continue
        fns.discard(mybir.ActivationFunctionType.Exp)
        fns.discard(mybir.ActivationFunctionType.Ln)
    return t


_hw_specs.get_activation_tables = _patched_get_tables
_bacc_mod.get_activation_tables = _patched_get_tables

import os
STOP_AFTER = os.environ.get("K_STOP_AFTER", "full")
REPS = int(os.environ.get("K_REPS", "1"))
# InstTensorTensorReduce crashes the exec unit on this toolchain/HW
USE_TTR = os.environ.get("K_USE_TTR", "0") == "1"


def expand_ap(ap, pattern, offset_elems=0):
    """AP over `ap`'s tensor keeping its partition dim, with explicit free
    [stride, count] pairs (strides in elements, outer->inner)."""
    new = [ap.ap[0]] + [[s, c] for (s, c) in pattern]
    return bass.AP(tensor=ap.tensor, offset=ap.offset + offset_elems, ap=new)


def build_program():
    nc = bacc.Bacc("TRN2", target_bir_lowering=False, debug=False,
                   num_devices=NCORES)

    d_ray0 = nc.dram_tensor("ray0", [R, 3], F32, kind="ExternalInput").ap()
    d_rd = nc.dram_tensor("rd", [R, 3], F32, kind="ExternalInput").ap()
    d_W1 = nc.dram_tensor("W1", [3, H], F32, kind="ExternalInput").ap()
    d_b1 = nc.dram_tensor("b1", [H], F32, kind="ExternalInput").ap()
    d_W2 = nc.dram_tensor("W2", [H, H], F32, kind="ExternalInput").ap()
    d_b2 = nc.dram_tensor("b2", [H], F32, kind="ExternalInput").ap()
    d_W3 = nc.dram_tensor("W3", [H, H], F32, kind="ExternalInput").ap()
    d_b3 = nc.dram_tensor("b3", [H], F32, kind="ExternalInput").ap()
    d_W4 = nc.dram_tensor("W4", [H, 1], F32, kind="ExternalInput").ap()
    d_b4 = nc.dram_tensor("b4", [1], F32, kind="ExternalInput").ap()
    d_t128 = nc.dram_tensor("t128", [128, S], F32, kind="ExternalInput").ap()
    d_wcost = nc.dram_tensor("wcost", [128, S], F32, kind="ExternalInput").ap()
    d_ident = nc.dram_tensor("ident", [128, 128], F32, kind="ExternalInput").ap()
    d_out = nc.dram_tensor("out", [R], F32, kind="ExternalOutput").ap()

    with tile.TileContext(nc) as tc:
        _emit(nc, tc, d_ray0, d_rd, d_W1, d_b1, d_W2, d_b2, d_W3, d_b3,
              d_W4, d_b4, d_t128, d_wcost, d_ident, d_out)

    nc.compile()
    return nc


def _emit(nc, tc, d_ray0, d_rd, d_W1, d_b1, d_W2, d_b2, d_W3, d_b3,
          d_W4, d_b4, d_t128, d_wcost, d_ident, d_out):
    ctx = ExitStack()
    sing = ctx.enter_context(tc.tile_pool(name="sing", bufs=1))
    work = ctx.enter_context(tc.tile_pool(name="work", bufs=2))
    z1p = ctx.enter_context(tc.tile_pool(name="z1p", bufs=2))
    ep = ctx.enter_context(tc.tile_pool(name="ep", bufs=2))
    fin = ctx.enter_context(tc.tile_pool(name="fin", bufs=1))
    bcp = ctx.enter_context(tc.tile_pool(name="bcp", bufs=2))
    rowp = ctx.enter_context(tc.tile_pool(name="rowp", bufs=3))
    psA = ctx.enter_context(tc.tile_pool(name="psA", bufs=3, space="PSUM"))
    psZ4 = ctx.enter_context(tc.tile_pool(name="psZ4", bufs=1, space="PSUM"))
    dram = ctx.enter_context(tc.tile_pool(name="dram", bufs=1, space="DRAM"))

    # ================= constants & weights =================
    t128 = sing.tile([128, S], F32)        # t128[p, s] = t[s]
    wcost = sing.tile([128, S], F32)       # sign-scan cost weights
    ident = sing.tile([128, 128], F32)
    nc.sync.dma_start(out=t128[:], in_=d_t128)
    nc.sync.dma_start(out=wcost[:], in_=d_wcost)
    nc.sync.dma_start(out=ident[:], in_=d_ident)

    w2 = [[sing.tile([128, 128], F32R, tag=f"w2_{k}_{c}", name=f"w2_{k}_{c}")
           for c in range(2)] for k in range(2)]
    w3 = [[sing.tile([128, 128], F32R, tag=f"w3_{k}_{c}", name=f"w3_{k}_{c}")
           for c in range(2)] for k in range(2)]
    w4 = [sing.tile([128, 1], F32R, tag=f"w4_{k}", name=f"w4_{k}")
          for k in range(2)]
    w1 = [sing.tile([3, 128], F32R, tag=f"w1_{c}", name=f"w1_{c}")
          for c in range(2)]
    for k in range(2):
        for c in range(2):
            nc.sync.dma_start(out=w2[k][c][:],
                              in_=d_W2[k * 128:(k + 1) * 128,
                                       c * 128:(c + 1) * 128].bitcast(F32R))
            nc.sync.dma_start(out=w3[k][c][:],
                              in_=d_W3[k * 128:(k + 1) * 128,
                                       c * 128:(c + 1) * 128].bitcast(F32R))
        nc.sync.dma_start(out=w4[k][:],
                          in_=d_W4[k * 128:(k + 1) * 128, :].bitcast(F32R))
    for c in range(2):
        nc.sync.dma_start(out=w1[c][:],
                          in_=d_W1[:, c * 128:(c + 1) * 128].bitcast(F32R))

    b1c = [sing.tile([128, 1], F32, tag=f"b1_{c}", name=f"b1_{c}") for c in range(2)]
    b2c = [sing.tile([128, 1], F32, tag=f"b2_{c}", name=f"b2_{c}") for c in range(2)]
    b3c = [sing.tile([128, 1], F32, tag=f"b3_{c}", name=f"b3_{c}") for c in range(2)]
    for c in range(2):
        nc.sync.dma_start(out=b1c[c][:], in_=d_b1[c * 128:(c + 1) * 128]
                          .rearrange("(h o) -> h o", o=1))
        nc.sync.dma_start(out=b2c[c][:], in_=d_b2[c * 128:(c + 1) * 128]
                          .rearrange("(h o) -> h o", o=1))
        nc.sync.dma_start(out=b3c[c][:], in_=d_b3[c * 128:(c + 1) * 128]
                          .rearrange("(h o) -> h o", o=1))
    b4col = sing.tile([128, 1], F32)
    nc.sync.dma_start(out=b4col[:],
                      in_=bass.AP(tensor=d_b4.tensor, offset=d_b4.offset,
                                  ap=[[0, 128], [1, 1]]))

    r0T = sing.tile([3, R], F32R)
    rdT = sing.tile([3, R], F32R)
    nc.sync.dma_start(out=r0T[:], in_=d_ray0.rearrange("n c -> c n").bitcast(F32R))
    nc.sync.dma_start(out=rdT[:], in_=d_rd.rearrange("n c -> c n").bitcast(F32R))

    # ================= A/C layer-1 folds =================
    # A[c][f, ray] = (W1^T ray0^T)[f, ray] + b1[f];  C[c] = W1^T rd^T
    A = [sing.tile([128, R], F32, tag=f"A_{c}", name=f"A_{c}") for c in range(2)]
    C = [sing.tile([128, R], F32, tag=f"C_{c}", name=f"C_{c}") for c in range(2)]
    for c in range(2):
        for h in range(2):
            ps = psA.tile([128, BP], F32, tag="z")
            nc.tensor.matmul(ps[:, 0:512], w1[c][:],
                             r0T[:, h * 512:(h + 1) * 512],
                             start=True, stop=True)
            nc.scalar.activation(A[c][:, h * 512:(h + 1) * 512], ps[:, 0:512],
                                 AF.Identity, bias=b1c[c][:])
            ps2 = psA.tile([128, BP], F32, tag="z")
            nc.tensor.matmul(ps2[:, 0:512], w1[c][:],
                             rdT[:, h * 512:(h + 1) * 512],
                             start=True, stop=True)
            nc.scalar.activation(C[c][:, h * 512:(h + 1) * 512], ps2[:, 0:512],
                                 AF.Identity)

    # ================= cube intersection =================
    statsd = dram.tile([NTILE, 2, 128], F32)   # [tile][dn_sel; g_sel][ray]
    bigt = sing.tile([128, 6], F32)
    nc.vector.memset(bigt[:], BIG)
    c1e9 = sing.tile([128, 3], F32)
    nc.vector.memset(c1e9[:], 1e-9)
    c1em12 = sing.tile([128, NTILE], F32)
    nc.vector.memset(c1em12[:], 1e-12)

    for p in range(NTILE):
        r0t = work.tile([128, 3], F32, tag="r0t")
        rdt = work.tile([128, 3], F32, tag="rdt")
        nc.sync.dma_start(out=r0t[:], in_=d_ray0[p * 128:(p + 1) * 128, :])
        nc.sync.dma_start(out=rdt[:], in_=d_rd[p * 128:(p + 1) * 128, :])

        # den = where(|rd| < 1e-9, 1e-9, rd)
        den = work.tile([128, 3], F32, tag="den")
        nc.vector.tensor_scalar(den[:].bitcast(U32), rdt[:].bitcast(U32),
                                0x7FFFFFFF, None, ALU.bitwise_and)
        msmall = work.tile([128, 3], F32, tag="msmall")
        nc.vector.tensor_scalar(msmall[:], den[:], 1e-9, None, ALU.is_lt)
        nc.vector.tensor_copy(den[:], rdt[:])
        nc.vector.copy_predicated(den[:], msmall[:].bitcast(U32), c1e9[:])
        inv = work.tile([128, 3], F32, tag="inv")
        nc.vector.reciprocal(inv[:], den[:])

        # d6 = (plane - ray0) * inv, planes (+p,+p,+p,-p,-p,-p)
        d6 = work.tile([128, 6], F32, tag="d6")
        nom = work.tile([128, 3], F32, tag="nom")
        nc.vector.tensor_scalar(nom[:], r0t[:], -1.0, P_DIST, ALU.mult, ALU.add)
        nc.vector.tensor_mul(d6[:, 0:3], nom[:], inv[:])
        nc.vector.tensor_scalar(nom[:], r0t[:], -1.0, -P_DIST, ALU.mult, ALU.add)
        nc.vector.tensor_mul(d6[:, 3:6], nom[:], inv[:])

        # p_int[r, k, j] = ray0[r, j] + d6[r, k] * rd[r, j]
        pi = work.tile([128, 18], F32, tag="pi")
        nc.vector.tensor_tensor(pi[:], expand_ap(d6[:], [(1, 6), (0, 3)]),
                                expand_ap(rdt[:], [(0, 6), (1, 3)]), ALU.mult)
        nc.vector.tensor_tensor(pi[:], pi[:],
                                expand_ap(r0t[:], [(0, 6), (1, 3)]), ALU.add)
        nc.vector.tensor_scalar(pi[:].bitcast(U32), pi[:].bitcast(U32),
                                0x7FFFFFFF, None, ALU.bitwise_and)
        nc.vector.tensor_scalar(pi[:], pi[:], P_DIST + EPS, None, ALU.is_le)
        in6 = work.tile([128, 6], F32, tag="in6")
        nc.vector.tensor_reduce(in6[:], pi[:].rearrange("p (k j) -> p k j", j=3),
                                axis=AX.X, op=ALU.min)
        cnt = work.tile([128, 1], F32, tag="cnt")
        nc.vector.tensor_reduce(cnt[:], in6[:], axis=AX.X, op=ALU.add)
        mcube = work.tile([128, 1], F32, tag="mcube")
        nc.vector.tensor_scalar(mcube[:], cnt[:], 2.0, None, ALU.is_equal)

        # dn = min inside |d|, df = max inside |d|
        nc.vector.tensor_scalar(d6[:].bitcast(U32), d6[:].bitcast(U32),
                                0x7FFFFFFF, None, ALU.bitwise_and)
        lo6 = work.tile([128, 6], F32, tag="lo6")
        nc.vector.tensor_copy(lo6[:], bigt[:])
        nc.vector.copy_predicated(lo6[:], in6[:].bitcast(U32), d6[:])
        dn_r = work.tile([128, 1], F32, tag="dn_r")
        nc.vector.tensor_reduce(dn_r[:], lo6[:], axis=AX.X, op=ALU.min)
        hi6 = work.tile([128, 6], F32, tag="hi6")
        nc.vector.tensor_mul(hi6[:], in6[:], d6[:])
        df_r = work.tile([128, 1], F32, tag="df_r")
        nc.vector.tensor_reduce(df_r[:], hi6[:], axis=AX.X, op=ALU.max)

        # st2 = [mcube*dn, mcube*(df-dn-2.4)+2.4]
        st2 = work.tile([128, 2], F32, tag="st2")
        g_r = work.tile([128, 1], F32, tag="g_r")
        nc.vector.tensor_sub(g_r[:], df_r[:], dn_r[:])
        nc.vector.tensor_mul(st2[:, 0:1], mcube[:], dn_r[:])
        nc.vector.tensor_scalar(g_r[:], g_r[:], 1.0, -(DEPTH_HI - DEPTH_LO),
                                ALU.mult, ALU.add)
        nc.vector.tensor_mul(g_r[:], g_r[:], mcube[:])
        nc.vector.tensor_scalar(st2[:, 1:2], g_r[:], 1.0, DEPTH_HI - DEPTH_LO,
                                ALU.mult, ALU.add)

        pst = psZ4.tile([128, BP], F32, tag="z4")
        nc.tensor.transpose(pst[0:2, 0:128], st2[:], ident[:])
        strow = work.tile([2, 128], F32, tag="strow")
        nc.vector.tensor_copy(strow[:], pst[0:2, 0:128])
        nc.sync.dma_start(out=statsd[p, :, :], in_=strow[:])

    # (tile,k)-partition layouts for the proposal-depth construction
    dn128 = sing.tile([128, NTILE], F32)
    g128 = sing.tile([128, NTILE], F32)
    for p in range(NTILE):
        nc.sync.dma_start(out=dn128[p * 16:(p + 1) * 16, :],
                          in_=statsd[p, 0, :].rearrange("(k r) -> k r", r=8))
        nc.sync.dma_start(out=g128[p * 16:(p + 1) * 16, :],
                          in_=statsd[p, 1, :].rearrange("(k r) -> k r", r=8))
    # (ray,tile) layouts for the scan / secant math
    st_dnc = sing.tile([128, NTILE], F32)
    st_gc = sing.tile([128, NTILE], F32)
    nc.sync.dma_start(out=st_dnc[:], in_=statsd[:, 0, :].rearrange("p r -> r p"))
    nc.sync.dma_start(out=st_gc[:], in_=statsd[:, 1, :].rearrange("p r -> r p"))

    if STOP_AFTER == "cube":
        nc.sync.dma_start(out=d_out.rearrange("(t r) -> r t", r=128),
                          in_=st_dnc[:])
        ctx.close()
        return

    # ================= proposal depths D8b =================
    # D8b[(p,k), (r8, s)] = dn128[(p,k), r8] + t[s] * g128[(p,k), r8]
    D8b = sing.tile([128, BP], F32)
    nc.vector.tensor_tensor(D8b[:], expand_ap(t128[:], [(0, 8), (1, S)]),
                            expand_ap(g128[:], [(1, 8), (0, S)]), ALU.mult)
    nc.vector.tensor_tensor(D8b[:], D8b[:],
                            expand_ap(dn128[:], [(1, 8), (0, S)]), ALU.add)

    vald = dram.tile([NB, BP], F32)     # stage-A logits, flat batch rows
    d8d = dram.tile([NB, BP], F32)      # proposal depths, flat batch rows
    nc.sync.dma_start(out=d8d[:], in_=D8b[:])

    # ================= MLP forward for one 1024-point batch ==============
    def mlp_batch(db_ap, per_ray, want_sig, tag_final=False):
        """db_ap: [128, 1024] depth broadcast. per_ray=(p0,k0) selects
        stage-A per-ray scalars; None means point index == ray index.
        Returns (z4 psum tile, e_list or None)."""
        z1 = z1p.tile([128, 2 * BP], F32, tag="z1")
        if per_ray is not None:
            p0, k0 = per_ray
            g0 = p0 * 128 + k0 * 8
            for c in range(2):
                for j in range(8):
                    nc.vector.tensor_scalar(
                        z1[:, c * BP + j * 128:c * BP + (j + 1) * 128],
                        db_ap[:, j * 128:(j + 1) * 128],
                        C[c][:, g0 + j:g0 + j + 1],
                        A[c][:, g0 + j:g0 + j + 1],
                        ALU.mult, ALU.add)
        else:
            for c in range(2):
                nc.vector.tensor_mul(z1[:, c * BP:(c + 1) * BP], db_ap, C[c][:])
                nc.vector.tensor_add(z1[:, c * BP:(c + 1) * BP],
                                     z1[:, c * BP:(c + 1) * BP], A[c][:])

        e1 = ep.tile([128, 2 * BP], F32, tag="e", name="e1", bufs=3)
        nc.scalar.activation(e1[:], z1[:], AF.Exp)
        sigs = []
        if want_sig:
            s1 = fin.tile([128, 2 * BP], F32, tag="s", name="s1", bufs=3)
            nc.scalar.activation(s1[:], z1[:], AF.Sigmoid)
            sigs.append(s1)
        h1 = ep.tile([128, 2 * BP], F32R, tag="h", name="h1", bufs=4)
        nc.scalar.activation(h1[:], e1[:], AF.Ln, bias=1.0)

        def layer(h_in, wk, bk, tagl):
            e = ep.tile([128, 2 * BP], F32, tag="e", name="e" + tagl, bufs=3)
            if want_sig:
                sl = fin.tile([128, 2 * BP], F32, tag="s", name="s" + tagl, bufs=3)
            for c in range(2):
                ps = psA.tile([128, BP], F32, tag="z")
                for k in range(2):
                    for hh in range(2):
                        nc.tensor.matmul(
                            ps[:, hh * 512:(hh + 1) * 512], wk[k][c][:],
                            h_in[:, k * BP + hh * 512:k * BP + (hh + 1) * 512],
                            start=(k == 0), stop=(k == 1))
                nc.scalar.activation(e[:, c * BP:(c + 1) * BP], ps[:],
                                     AF.Exp, bias=bk[c][:])
                if want_sig:
                    nc.scalar.activation(sl[:, c * BP:(c + 1) * BP], ps[:],
                                         AF.Sigmoid, bias=bk[c][:])
            if want_sig:
                sigs.append(sl)
            h = ep.tile([128, 2 * BP], F32R, tag="h", name="h" + tagl, bufs=4)
            nc.scalar.activation(h[:], e[:], AF.Ln, bias=1.0)
            return h

        h2 = layer(h1, w2, b2c, "2")
        h3 = layer(h2, w3, b3c, "3")
        z4 = psZ4.tile([128, BP], F32, tag="z4")
        for k in range(2):
            for hh in range(2):
                nc.tensor.matmul(z4[0:1, hh * 512:(hh + 1) * 512], w4[k][:],
                                 h3[:, k * BP + hh * 512:k * BP + (hh + 1) * 512],
                                 start=(k == 0), stop=(k == 1))
        return z4, (sigs if want_sig else None)

    def z4_to_dram(z4, dst_row_ap):
        row = rowp.tile([1, BP], F32, tag="z4row", name="row")
        nc.vector.tensor_copy(row[:], z4[0:1, :])
        nc.sync.dma_start(out=dst_row_ap, in_=row[:])

    # ================= stage A (3-deep software pipeline) =================
    # front(b): depth bcast + layer-1 FMA + Exp/Ln -> h1
    # mid(b):   layer-2 matmuls + Exp/Ln -> h2
    # back(b):  layer-3 matmuls + Exp/Ln -> h3, z4 matmuls, row copy to DRAM
    live = {}

    def stage_front(b):
        p0, k0 = b // 16, b % 16
        g0 = p0 * 128 + k0 * 8
        drow = rowp.tile([1, BP], F32, tag="drow", name="drow")
        nc.sync.dma_start(out=drow[:], in_=d8d[b:b + 1, :])
        db = bcp.tile([128, BP], F32, tag="db", name="db")
        nc.gpsimd.partition_broadcast(db[:], drow[:])
        z1 = z1p.tile([128, 2 * BP], F32, tag="z1", name="z1")
        for c in range(2):
            for j in range(8):
                nc.vector.tensor_scalar(
                    z1[:, c * BP + j * 128:c * BP + (j + 1) * 128],
                    db[:, j * 128:(j + 1) * 128],
                    C[c][:, g0 + j:g0 + j + 1],
                    A[c][:, g0 + j:g0 + j + 1],
                    ALU.mult, ALU.add)
        e1 = ep.tile([128, 2 * BP], F32, tag="e", name="e1", bufs=3)
        nc.scalar.activation(e1[:], z1[:], AF.Exp)
        h1 = ep.tile([128, 2 * BP], F32R, tag="h", name="h1", bufs=4)
        nc.scalar.activation(h1[:], e1[:], AF.Ln, bias=1.0)
        live[b] = {"h1": h1}

    def _mm_layer(h_in, wk, bk, nm):
        e = ep.tile([128, 2 * BP], F32, tag="e", name="e" + nm, bufs=3)
        for c in range(2):
            ps = psA.tile([128, BP], F32, tag="z", name="ps" + nm)
            for k in range(2):
                for hh in range(2):
                    nc.tensor.matmul(
                        ps[:, hh * 512:(hh + 1) * 512], wk[k][c][:],
                        h_in[:, k * BP + hh * 512:k * BP + (hh + 1) * 512],
                        start=(k == 0), stop=(k == 1))
            nc.scalar.activation(e[:, c * BP:(c + 1) * BP], ps[:],
                                 AF.Exp, bias=bk[c][:])
        h = ep.tile([128, 2 * BP], F32R, tag="h", name="h" + nm, bufs=4)
        nc.scalar.activation(h[:], e[:], AF.Ln, bias=1.0)
        return h

    def stage_mid(b):
        live[b]["h2"] = _mm_layer(live[b].pop("h1"), w2, b2c, "2")

    def stage_back(b):
        h3 = _mm_layer(live[b].pop("h2"), w3, b3c, "3")
        z4 = psZ4.tile([128, BP], F32, tag="z4", name="z4A")
        for k in range(2):
            for hh in range(2):
                nc.tensor.matmul(z4[0:1, hh * 512:(hh + 1) * 512], w4[k][:],
                                 h3[:, k * BP + hh * 512:k * BP + (hh + 1) * 512],
                                 start=(k == 0), stop=(k == 1))
        z4_to_dram(z4, vald[b:b + 1, :])
        del live[b]

    for _rep in range(REPS):
        for i in range(NB + 2):
            if i < NB:
                stage_front(i)
            if 1 <= i and i - 1 < NB:
                stage_mid(i - 1)
            if 2 <= i and i - 2 < NB:
                stage_back(i - 2)

    if STOP_AFTER == "stageA":
        tmpo = work.tile([128, NTILE], F32, tag="tmpo")
        for p in range(NTILE):
            nc.sync.dma_start(
                out=tmpo[:, p:p + 1],
                in_=vald[p * 16:(p + 1) * 16, :].rearrange(
                    "k (r s) -> (k r) s", s=S)[:, 0:1])
        nc.sync.dma_start(out=d_out.rearrange("(t r) -> r t", r=128),
                          in_=tmpo[:])
        ctx.close()
        return

    # ================= scan =================
    st_dlow = sing.tile([128, NTILE], F32)
    st_flow = sing.tile([128, NTILE], F32)
    st_dhigh = sing.tile([128, NTILE], F32)
    st_fhigh = sing.tile([128, NTILE], F32)
    st_mask = sing.tile([128, NTILE], F32)

    for p in range(NTILE):
        val = work.tile([128, S], F32, tag="val")
        nc.sync.dma_start(
            out=val[:],
            in_=vald[p * 16:(p + 1) * 16, :].rearrange(
                "k (r s) -> (k r) s", s=S))
        nc.vector.tensor_scalar(val[:], val[:], b4col[:], -LOGIT_TAU,
                                ALU.add, ALU.add)

        prod = work.tile([128, S], F32, tag="prod")
        nc.vector.tensor_tensor(prod[:, 0:S - 1], val[:, 0:S - 1],
                                val[:, 1:S], ALU.mult)
        sgn = work.tile([128, S], F32, tag="sgn")
        nc.scalar.activation(sgn[:, 0:S - 1], prod[:, 0:S - 1], AF.Sign)
        nc.vector.memset(sgn[:, S - 1:S], 1.0)
        cost = work.tile([128, S], F32, tag="cost")
        nc.vector.tensor_tensor(cost[:], sgn[:], wcost[:], ALU.mult)
        vmin = work.tile([128, 1], F32, tag="vmin")
        nc.vector.tensor_reduce(vmin[:], cost[:], axis=AX.X, op=ALU.min)
        oh = work.tile([128, S], F32, tag="oh")
        nc.vector.tensor_scalar(oh[:], cost[:], vmin[:], None, ALU.is_equal)
        ohh = work.tile([128, S], F32, tag="ohh")
        nc.vector.memset(ohh[:, 0:1], 0.0)
        nc.vector.tensor_copy(ohh[:, 1:S], oh[:, 0:S - 1])
        nc.vector.tensor_add(ohh[:, S - 1:S], ohh[:, S - 1:S], oh[:, S - 1:S])

        scr = work.tile([128, S], F32, tag="scr")
        f_low = work.tile([128, 1], F32, tag="f_low")
        f_high = work.tile([128, 1], F32, tag="f_high")
        t_low = work.tile([128, 1], F32, tag="t_low")
        t_high = work.tile([128, 1], F32, tag="t_high")
        if USE_TTR:
            nc.vector.tensor_tensor_reduce(scr[:], val[:], oh[:], 1.0, 0.0,
                                           ALU.mult, ALU.add, f_low[:])
            nc.vector.tensor_tensor_reduce(scr[:], val[:], ohh[:], 1.0, 0.0,
                                           ALU.mult, ALU.add, f_high[:])
            nc.vector.tensor_tensor_reduce(scr[:], t128[:], oh[:], 1.0, 0.0,
                                           ALU.mult, ALU.add, t_low[:])
            nc.vector.tensor_tensor_reduce(scr[:], t128[:], ohh[:], 1.0, 0.0,
                                           ALU.mult, ALU.add, t_high[:])
        else:
            nc.vector.tensor_mul(scr[:], val[:], oh[:])
            nc.vector.tensor_reduce(f_low[:], scr[:], axis=AX.X, op=ALU.add)
            nc.vector.tensor_mul(scr[:], val[:], ohh[:])
            nc.vector.tensor_reduce(f_high[:], scr[:], axis=AX.X, op=ALU.add)
            nc.vector.tensor_mul(scr[:], t128[:], oh[:])
            nc.vector.tensor_reduce(t_low[:], scr[:], axis=AX.X, op=ALU.add)
            nc.vector.tensor_mul(scr[:], t128[:], ohh[:])
            nc.vector.tensor_reduce(t_high[:], scr[:], axis=AX.X, op=ALU.add)

        nc.vector.tensor_mul(t_low[:], t_low[:], st_gc[:, p:p + 1])
        nc.vector.tensor_add(st_dlow[:, p:p + 1], t_low[:], st_dnc[:, p:p + 1])
        nc.vector.tensor_mul(t_high[:], t_high[:], st_gc[:, p:p + 1])
        nc.vector.tensor_add(st_dhigh[:, p:p + 1], t_high[:],
                             st_dnc[:, p:p + 1])
        nc.vector.tensor_copy(st_flow[:, p:p + 1], f_low[:])
        nc.vector.tensor_copy(st_fhigh[:, p:p + 1], f_high[:])

        m1 = work.tile([128, 1], F32, tag="m1")
        nc.vector.tensor_scalar(m1[:], vmin[:], 0.0, None, ALU.is_lt)
        m2 = work.tile([128, 1], F32, tag="m2")
        nc.vector.tensor_scalar(m2[:], f_low[:], 0.0, None, ALU.is_lt)
        nc.vector.tensor_mul(m1[:], m1[:], m2[:])
        nc.vector.tensor_scalar(m2[:], val[:, 0:1], 0.0, None, ALU.is_lt)
        nc.vector.tensor_mul(st_mask[:, p:p + 1], m1[:], m2[:])

    if STOP_AFTER == "scan":
        nc.sync.dma_start(out=d_out.rearrange("(t r) -> r t", r=128),
                          in_=st_dlow[:])
        ctx.close()
        return

    # ================= secant =================
    st_dpred = sing.tile([128, NTILE], F32)

    def secant_dpred():
        den = work.tile([128, NTILE], F32, tag="sden")
        nc.vector.tensor_sub(den[:], st_fhigh[:], st_flow[:])
        dabs = work.tile([128, NTILE], F32, tag="sdabs")
        nc.vector.tensor_scalar(dabs[:].bitcast(U32), den[:].bitcast(U32),
                                0x7FFFFFFF, None, ALU.bitwise_and)
        msk = work.tile([128, NTILE], F32, tag="smsk")
        nc.vector.tensor_scalar(msk[:], dabs[:], 1e-12, None, ALU.is_lt)
        nc.vector.copy_predicated(den[:], msk[:].bitcast(U32), c1em12[:])
        rec = work.tile([128, NTILE], F32, tag="srec")
        nc.vector.reciprocal(rec[:], den[:])
        num = work.tile([128, NTILE], F32, tag="snum")
        nc.vector.tensor_sub(num[:], st_dhigh[:], st_dlow[:])
        nc.vector.tensor_mul(num[:], num[:], st_flow[:])
        nc.vector.tensor_mul(num[:], num[:], rec[:])
        nc.vector.tensor_sub(st_dpred[:], st_dlow[:], num[:])

    secant_dpred()

    dpredd = dram.tile([NSEC + 1, R], F32)   # flattened d_pred per use
    fmd = dram.tile([NSEC + 1, R], F32)      # flattened f / df rows

    def eval_f(i, want_sig=False, tag_final=False):
        nc.sync.dma_start(out=dpredd[i, :].rearrange("(t r) -> r t", r=128),
                          in_=st_dpred[:])
        dfl = rowp.tile([1, R], F32, tag="dflat", name="dfl")
        nc.sync.dma_start(out=dfl[:], in_=dpredd[i:i + 1, :])
        db = bcp.tile([128, R], F32, tag="db")
        nc.gpsimd.partition_broadcast(db[:], dfl[:])
        z4, es = mlp_batch(db[:], None, want_sig, tag_final)
        z4_to_dram(z4, fmd[i:i + 1, :])
        fm = work.tile([128, NTILE], F32, tag="fm")
        nc.sync.dma_start(out=fm[:],
                          in_=fmd[i, :].rearrange("(t r) -> r t", r=128))
        nc.vector.tensor_scalar(fm[:], fm[:], b4col[:], -LOGIT_TAU,
                                ALU.add, ALU.add)
        return fm, db, es

    for it in range(NSEC):
        fm, _, _ = eval_f(it)
        low = work.tile([128, NTILE], F32, tag="lowm")
        nc.vector.tensor_scalar(low[:], fm[:], 0.0, None, ALU.is_lt)
        nc.vector.copy_predicated(st_dlow[:], low[:].bitcast(U32), st_dpred[:])
        nc.vector.copy_predicated(st_flow[:], low[:].bitcast(U32), fm[:])
        hi = work.tile([128, NTILE], F32, tag="him")
        nc.vector.tensor_scalar(hi[:], low[:], -1.0, 1.0, ALU.mult, ALU.add)
        nc.vector.copy_predicated(st_dhigh[:], hi[:].bitcast(U32), st_dpred[:])
        nc.vector.copy_predicated(st_fhigh[:], hi[:].bitcast(U32), fm[:])
        secant_dpred()

    if STOP_AFTER == "secant":
        nc.sync.dma_start(out=d_out.rearrange("(t r) -> r t", r=128),
                          in_=st_dpred[:])
        ctx.close()
        return

    # ================= final newton correction =================
    fval, dbF, sigs = eval_f(NSEC, want_sig=True, tag_final=True)
    s1, s2, s3 = sigs

    dh1 = fin.tile([128, 2 * BP], F32R, tag="dh", name="dh1", bufs=2)
    for c in range(2):
        nc.vector.tensor_mul(dh1[:, c * BP:(c + 1) * BP],
                             s1[:, c * BP:(c + 1) * BP], C[c][:])

    def tangent_layer(dh_in, wk, s_l, tagl):
        dh = fin.tile([128, 2 * BP], F32R, tag="dh", name="dh", bufs=2)
        for c in range(2):
            ps = psA.tile([128, BP], F32, tag="z")
            for k in range(2):
                for hh in range(2):
                    nc.tensor.matmul(
                        ps[:, hh * 512:(hh + 1) * 512], wk[k][c][:],
                        dh_in[:, k * BP + hh * 512:k * BP + (hh + 1) * 512],
                        start=(k == 0), stop=(k == 1))
            nc.vector.tensor_mul(dh[:, c * BP:(c + 1) * BP],
                                 s_l[:, c * BP:(c + 1) * BP], ps[:])
        return dh

    dh2 = tangent_layer(dh1, w2, s2, "2")
    dh3 = tangent_layer(dh2, w3, s3, "3")
    dfp = psZ4.tile([128, BP], F32, tag="z4")
    for k in range(2):
        for hh in range(2):
            nc.tensor.matmul(dfp[0:1, hh * 512:(hh + 1) * 512], w4[k][:],
                             dh3[:, k * BP + hh * 512:k * BP + (hh + 1) * 512],
                             start=(k == 0), stop=(k == 1))
    dfdd_d = dram.tile([1, R], F32)
    row = rowp.tile([1, BP], F32, tag="z4row", name="row")
    nc.vector.tensor_copy(row[:], dfp[0:1, :])
    nc.sync.dma_start(out=dfdd_d[:], in_=row[:])
    dfdd = work.tile([128, NTILE], F32, tag="dfdd")
    nc.sync.dma_start(out=dfdd[:],
                      in_=dfdd_d[0, :].rearrange("(t r) -> r t", r=128))

    # clamp: |df|<1e-6 -> sign(df)*1e-6 (df==0 -> +1e-6)
    dneg = work.tile([128, NTILE], F32, tag="dneg")
    nc.vector.tensor_scalar(dneg[:], dfdd[:], 0.0, None, ALU.is_lt)
    dabs = work.tile([128, NTILE], F32, tag="dfabs")
    nc.vector.tensor_scalar(dabs[:].bitcast(U32), dfdd[:].bitcast(U32),
                            0x7FFFFFFF, None, ALU.bitwise_and)
    nc.vector.tensor_scalar(dabs[:], dabs[:], 1e-6, None, ALU.max)
    nc.vector.tensor_scalar(dneg[:], dneg[:], -2.0, 1.0, ALU.mult, ALU.add)
    nc.vector.tensor_mul(dfdd[:], dabs[:], dneg[:])

    rec = work.tile([128, NTILE], F32, tag="recF")
    nc.vector.reciprocal(rec[:], dfdd[:])
    nc.vector.tensor_mul(fval[:], fval[:], rec[:])
    dout = work.tile([128, NTILE], F32, tag="dout")
    nc.vector.tensor_sub(dout[:], st_dpred[:], fval[:])
    nc.vector.tensor_mul(dout[:], dout[:], st_mask[:])

    nc.sync.dma_start(out=d_out.rearrange("(t r) -> r t", r=128), in_=dout[:])

    ctx.close()


# ======================= host-side driver =======================

def _host_constants():
    t = np.linspace(0.0, 1.0, S).astype(np.float32)
    t128 = np.ascontiguousarray(np.broadcast_to(t, (128, S)))
    w = np.arange(S, 0, -1, dtype=np.float32)   # S, S-1, ..., 1
    wcost = np.ascontiguousarray(np.broadcast_to(w, (128, S)))
    ident = np.eye(128, dtype=np.float32)
    return t128, wcost, ident


def _make_runner(nc):
    """Persistent 8-core PJRT runner (mirrors bass2jax.run_bass_via_pjrt's
    multi-core path, but keeps the compiled shard_map callable across calls)."""
    import jax
    from jax.sharding import Mesh, PartitionSpec
    from jax.experimental.shard_map import shard_map
    from concourse import bass2jax
    from concourse import mybir as _mb

    bass2jax.install_neuronx_cc_hook()
    partition_name = (nc.partition_id_tensor.name
                      if nc.partition_id_tensor else None)
    in_names, out_names, out_avals, zero_shapes = [], [], [], []
    for alloc in nc.m.functions[0].allocations:
        if not isinstance(alloc, _mb.MemoryLocationSet):
            continue
        name = alloc.memorylocations[0].name
        if alloc.kind == "ExternalInput":
            if name != partition_name:
                in_names.append(name)
        elif alloc.kind == "ExternalOutput":
            out_names.append(name)
            shape = tuple(alloc.tensor_shape)
            dtype = _mb.dt.np(alloc.dtype)
            out_avals.append(jax.core.ShapedArray(shape, dtype))
            zero_shapes.append((shape, dtype))
    n_params = len(in_names)
    n_outs = len(out_avals)
    all_names = list(in_names) + list(out_names)
    if partition_name is not None:
        all_names.append(partition_name)
    donate = tuple(range(n_params, n_params + n_outs))

    def _body(*args):
        operands = list(args)
        if partition_name is not None:
            operands.append(bass2jax.partition_id_tensor())
        outs = bass2jax._bass_exec_p.bind(
            *operands,
            out_avals=tuple(out_avals),
            in_names=tuple(all_names),
            out_names=tuple(out_names),
            lowering_input_output_aliases=(),
            sim_require_finite=True,
            sim_require_nnan=True,
            nc=nc,
        )
        return tuple(outs)

    devices = jax.devices()[:NCORES]
    mesh = Mesh(np.asarray(devices), ("core",))
    in_specs = (PartitionSpec("core"),) * (n_params + n_outs)
    out_specs = (PartitionSpec("core"),) * n_outs
    sharded = jax.jit(
        shard_map(_body, mesh=mesh, in_specs=in_specs, out_specs=out_specs,
                  check_rep=False),
        donate_argnums=donate, keep_unused=True)

    def run(in_maps):
        concat_in = [
            np.concatenate([np.asarray(in_maps[c][nm]) for c in range(NCORES)],
                           axis=0)
            for nm in in_names
        ]
        concat_zeros = [np.zeros((NCORES * sh[0], *sh[1:]), dt)
                        for (sh, dt) in zero_shapes]
        outs = jax.block_until_ready(sharded(*concat_in, *concat_zeros))
        return [
            {nm: np.asarray(outs[i]).reshape(NCORES, *out_avals[i].shape)[c]
             for i, nm in enumerate(out_names)}
            for c in range(NCORES)
        ]

    return run


def kernel(ray0, ray_direction, W1, b1, W2, b2, W3, b3, W4, b4):
    if "prog" not in _nc_cache:
        _nc_cache["prog"] = build_program()
        _nc_cache["runner"] = _make_runner(_nc_cache["prog"])
    nc = _nc_cache["prog"]

    t128, wcost, ident = _host_constants()
    r0 = np.ascontiguousarray(np.asarray(ray0).reshape(N, 3), np.float32)
    rd = np.ascontiguousarray(np.asarray(ray_direction).reshape(N, 3),
                              np.float32)
    shared = {
        "W1": np.ascontiguousarray(W1, np.float32),
        "b1": np.ascontiguousarray(b1, np.float32),
        "W2": np.ascontiguousarray(W2, np.float32),
        "b2": np.ascontiguousarray(b2, np.float32),
        "W3": np.ascontiguousarray(W3, np.float32),
        "b3": np.ascontiguousarray(b3, np.float32),
        "W4": np.ascontiguousarray(W4, np.float32),
        "b4": np.ascontiguousarray(b4, np.float32),
        "t128": t128, "wcost": wcost, "ident": ident,
    }
    in_maps = []
    for c in range(NCORES):
        m = dict(shared)
        m["ray0"] = r0[c * R:(c + 1) * R]
        m["rd"] = rd[c * R:(c + 1) * R]
        in_maps.append(m)

    results = _nc_cache["runner"](in_maps)
    out = np.concatenate([results[c]["out"] for c in range(NCORES)])
    return out.reshape(1, N).astype(np.float32)


# revision 26
# speedup vs baseline: 419.0192x; 150.6327x over previous
"""Trainium2 Bass kernel for nn_DepthModule: ray-marched implicit-surface depth.

kernel(**inputs) takes FULL unsharded inputs (ray0 [1,8192,3], ray_direction
[1,8192,3], MLP weights W1..b4) and returns the FULL output [1,8192] float32.
The N=8192 ray axis is sharded across 8 NeuronCores (data parallel, weights
replicated); each core runs an identical Bass/Tile program on its 1024 rays.

Per-core pipeline (mirrors the jax reference):
  1. cube entry/exit depths per ray (DVE vector math, rays on partitions)
  2. S=128 proposal depths/ray -> MLP occupancy logits for all 128K points
     (fp32r matmuls on PE; softplus(z) = Ln(Exp(z)+1) on the scalar engine,
     biases folded into Exp's pre-activation bias)
  3. first-sign-change scan (one-hot gather via tensor_tensor_reduce)
  4. 8 secant-refinement iterations (1024-point MLP evals)
  5. implicit-gradient Newton correction (forward + JVP tangent pass;
     sigmoid(z) = E/(E+1) recovered from the saved Exp values on DVE)
"""

from contextlib import ExitStack

import numpy as np

import concourse.bacc as bacc
import concourse.bass as bass
import concourse.tile as tile
import concourse.mybir as mybir
from concourse.bass_utils import run_bass_kernel_spmd  # noqa: F401 (fallback)

F32 = mybir.dt.float32
F32R = mybir.dt.float32r
U32 = mybir.dt.uint32
AF = mybir.ActivationFunctionType
ALU = mybir.AluOpType
AX = mybir.AxisListType

NCORES = 8
N = 8192
R = N // NCORES          # rays per core = 1024
S = 128                  # proposal samples per ray
H = 256                  # MLP hidden dim
NSEC = 8                 # secant iterations
NTILE = R // 128         # ray tiles per core = 8
BP = 1024                # points per stage-A batch (8 rays x 128 samples)
NB = R * S // BP         # stage-A batches = 128

TAU = 0.5
LOGIT_TAU = float(np.log(TAU / (1.0 - TAU)))   # 0.0
DEPTH_LO, DEPTH_HI = 0.0, 2.4
PADDING = 0.1
EPS = 1e-6
P_DIST = 0.5 + PADDING / 2.0
BIG = 1e9

_nc_cache = {}

# The act-table placement pass picks tables greedily per function; with Exp
# and Ln living in several tables it alternates exp_and_others/natural_log
# loads (~570 loads, 730us of ACT time). Restrict the bass-side view so
# natural_log_exp_and_others is the only table with Exp/Ln -> one load total.
# (Indices into act_info.json are preserved; walrus adopts the pre-placed
# load ids.)
import concourse.hw_specs as _hw_specs
import concourse.bacc as _bacc_mod

if getattr(_hw_specs.get_activation_tables, "_depth_patched", False):
    _orig_get_tables = _hw_specs.get_activation_tables._orig
else:
    _orig_get_tables = _hw_specs.get_activation_tables


def _patched_get_tables(arch):
    t = _orig_get_tables(arch)
    keep = "natural_log_exp_and_others"
    for name, fns in t.items():
        if name == keep:
            continue
        fns.discard(mybir.ActivationFunctionType.Exp)
        fns.discard(mybir.ActivationFunctionType.Ln)
    return t


_patched_get_tables._depth_patched = True
_patched_get_tables._orig = _orig_get_tables
_hw_specs.get_activation_tables = _patched_get_tables
_bacc_mod.get_activation_tables = _patched_get_tables

import os
REPS = int(os.environ.get("K_REPS", "1"))


def expand_ap(ap, pattern, offset_elems=0):
    """AP over `ap`'s tensor keeping its partition dim, with explicit free
    [stride, count] pairs (strides in elements, outer->inner)."""
    new = [ap.ap[0]] + [[s, c] for (s, c) in pattern]
    return bass.AP(tensor=ap.tensor, offset=ap.offset + offset_elems, ap=new)


def build_program():
    nc = bacc.Bacc("TRN2", target_bir_lowering=False, debug=False,
                   num_devices=NCORES)

    d_ray0 = nc.dram_tensor("ray0", [R, 3], F32, kind="ExternalInput").ap()
    d_rd = nc.dram_tensor("rd", [R, 3], F32, kind="ExternalInput").ap()
    d_W1 = nc.dram_tensor("W1", [3, H], F32, kind="ExternalInput").ap()
    d_b1 = nc.dram_tensor("b1", [H], F32, kind="ExternalInput").ap()
    d_W2 = nc.dram_tensor("W2", [H, H], F32, kind="ExternalInput").ap()
    d_b2 = nc.dram_tensor("b2", [H], F32, kind="ExternalInput").ap()
    d_W3 = nc.dram_tensor("W3", [H, H], F32, kind="ExternalInput").ap()
    d_b3 = nc.dram_tensor("b3", [H], F32, kind="ExternalInput").ap()
    d_W4 = nc.dram_tensor("W4", [H, 1], F32, kind="ExternalInput").ap()
    d_b4 = nc.dram_tensor("b4", [1], F32, kind="ExternalInput").ap()
    d_t128 = nc.dram_tensor("t128", [128, S], F32, kind="ExternalInput").ap()
    d_wcost = nc.dram_tensor("wcost", [128, S], F32, kind="ExternalInput").ap()
    d_ident = nc.dram_tensor("ident", [128, 128], F32, kind="ExternalInput").ap()
    d_out = nc.dram_tensor("out", [R], F32, kind="ExternalOutput").ap()

    with tile.TileContext(nc) as tc:
        _emit(nc, tc, d_ray0, d_rd, d_W1, d_b1, d_W2, d_b2, d_W3, d_b3,
              d_W4, d_b4, d_t128, d_wcost, d_ident, d_out)

    nc.compile()
    return nc


def _emit(nc, tc, d_ray0, d_rd, d_W1, d_b1, d_W2, d_b2, d_W3, d_b3,
          d_W4, d_b4, d_t128, d_wcost, d_ident, d_out):
    ctx = ExitStack()
    sing = ctx.enter_context(tc.tile_pool(name="sing", bufs=1))
    work = ctx.enter_context(tc.tile_pool(name="work", bufs=2))
    z1p = ctx.enter_context(tc.tile_pool(name="z1p", bufs=2))
    ep = ctx.enter_context(tc.tile_pool(name="ep", bufs=2))
    fin = ctx.enter_context(tc.tile_pool(name="fin", bufs=1))
    bcp = ctx.enter_context(tc.tile_pool(name="bcp", bufs=2))
    rowp = ctx.enter_context(tc.tile_pool(name="rowp", bufs=3))
    psA = ctx.enter_context(tc.tile_pool(name="psA", bufs=3, space="PSUM"))
    psZ4 = ctx.enter_context(tc.tile_pool(name="psZ4", bufs=1, space="PSUM"))
    dram = ctx.enter_context(tc.tile_pool(name="dram", bufs=1, space="DRAM"))

    # ================= constants & weights =================
    t128 = sing.tile([128, S], F32)        # t128[p, s] = t[s]
    wcost = sing.tile([128, S], F32)       # sign-scan cost weights
    ident = sing.tile([128, 128], F32)
    nc.sync.dma_start(out=t128[:], in_=d_t128)
    nc.sync.dma_start(out=wcost[:], in_=d_wcost)
    nc.sync.dma_start(out=ident[:], in_=d_ident)

    w2 = [[sing.tile([128, 128], F32R, tag=f"w2_{k}_{c}", name=f"w2_{k}_{c}")
           for c in range(2)] for k in range(2)]
    w3 = [[sing.tile([128, 128], F32R, tag=f"w3_{k}_{c}", name=f"w3_{k}_{c}")
           for c in range(2)] for k in range(2)]
    w4 = [sing.tile([128, 1], F32R, tag=f"w4_{k}", name=f"w4_{k}")
          for k in range(2)]
    w1 = [sing.tile([3, 128], F32R, tag=f"w1_{c}", name=f"w1_{c}")
          for c in range(2)]
    for k in range(2):
        for c in range(2):
            nc.sync.dma_start(out=w2[k][c][:],
                              in_=d_W2[k * 128:(k + 1) * 128,
                                       c * 128:(c + 1) * 128].bitcast(F32R))
            nc.sync.dma_start(out=w3[k][c][:],
                              in_=d_W3[k * 128:(k + 1) * 128,
                                       c * 128:(c + 1) * 128].bitcast(F32R))
        nc.sync.dma_start(out=w4[k][:],
                          in_=d_W4[k * 128:(k + 1) * 128, :].bitcast(F32R))
    for c in range(2):
        nc.sync.dma_start(out=w1[c][:],
                          in_=d_W1[:, c * 128:(c + 1) * 128].bitcast(F32R))

    b1c = [sing.tile([128, 1], F32, tag=f"b1_{c}", name=f"b1_{c}") for c in range(2)]
    b2c = [sing.tile([128, 1], F32, tag=f"b2_{c}", name=f"b2_{c}") for c in range(2)]
    b3c = [sing.tile([128, 1], F32, tag=f"b3_{c}", name=f"b3_{c}") for c in range(2)]
    for c in range(2):
        nc.sync.dma_start(out=b1c[c][:], in_=d_b1[c * 128:(c + 1) * 128]
                          .rearrange("(h o) -> h o", o=1))
        nc.sync.dma_start(out=b2c[c][:], in_=d_b2[c * 128:(c + 1) * 128]
                          .rearrange("(h o) -> h o", o=1))
        nc.sync.dma_start(out=b3c[c][:], in_=d_b3[c * 128:(c + 1) * 128]
                          .rearrange("(h o) -> h o", o=1))
    b4col = sing.tile([128, 1], F32)
    nc.sync.dma_start(out=b4col[:],
                      in_=bass.AP(tensor=d_b4.tensor, offset=d_b4.offset,
                                  ap=[[0, 128], [1, 1]]))

    r0T = sing.tile([3, R], F32R)
    rdT = sing.tile([3, R], F32R)
    nc.sync.dma_start(out=r0T[:], in_=d_ray0.rearrange("n c -> c n").bitcast(F32R))
    nc.sync.dma_start(out=rdT[:], in_=d_rd.rearrange("n c -> c n").bitcast(F32R))

    # ================= A/C layer-1 folds =================
    # A[c][f, ray] = (W1^T ray0^T)[f, ray] + b1[f];  C[c] = W1^T rd^T
    A = [sing.tile([128, R], F32, tag=f"A_{c}", name=f"A_{c}") for c in range(2)]
    C = [sing.tile([128, R], F32, tag=f"C_{c}", name=f"C_{c}") for c in range(2)]
    for c in range(2):
        for h in range(2):
            ps = psA.tile([128, BP], F32, tag="z")
            nc.tensor.matmul(ps[:, 0:512], w1[c][:],
                             r0T[:, h * 512:(h + 1) * 512],
                             start=True, stop=True)
            nc.scalar.activation(A[c][:, h * 512:(h + 1) * 512], ps[:, 0:512],
                                 AF.Identity, bias=b1c[c][:])
            ps2 = psA.tile([128, BP], F32, tag="z")
            nc.tensor.matmul(ps2[:, 0:512], w1[c][:],
                             rdT[:, h * 512:(h + 1) * 512],
                             start=True, stop=True)
            nc.scalar.activation(C[c][:, h * 512:(h + 1) * 512], ps2[:, 0:512],
                                 AF.Identity)

    # ================= cube intersection =================
    statsd = dram.tile([NTILE, 2, 128], F32)   # [tile][dn_sel; g_sel][ray]
    bigt = sing.tile([128, 6], F32)
    nc.vector.memset(bigt[:], BIG)
    c1e9 = sing.tile([128, 3], F32)
    nc.vector.memset(c1e9[:], 1e-9)
    c1em12 = sing.tile([128, NTILE], F32)
    nc.vector.memset(c1em12[:], 1e-12)

    for p in range(NTILE):
        r0t = work.tile([128, 3], F32, tag="r0t")
        rdt = work.tile([128, 3], F32, tag="rdt")
        nc.sync.dma_start(out=r0t[:], in_=d_ray0[p * 128:(p + 1) * 128, :])
        nc.sync.dma_start(out=rdt[:], in_=d_rd[p * 128:(p + 1) * 128, :])

        # den = where(|rd| < 1e-9, 1e-9, rd)
        den = work.tile([128, 3], F32, tag="den")
        nc.vector.tensor_scalar(den[:].bitcast(U32), rdt[:].bitcast(U32),
                                0x7FFFFFFF, None, ALU.bitwise_and)
        msmall = work.tile([128, 3], F32, tag="msmall")
        nc.vector.tensor_scalar(msmall[:], den[:], 1e-9, None, ALU.is_lt)
        nc.vector.tensor_copy(den[:], rdt[:])
        nc.vector.copy_predicated(den[:], msmall[:].bitcast(U32), c1e9[:])
        inv = work.tile([128, 3], F32, tag="inv")
        nc.vector.reciprocal(inv[:], den[:])

        # d6 = (plane - ray0) * inv, planes (+p,+p,+p,-p,-p,-p)
        d6 = work.tile([128, 6], F32, tag="d6")
        nom = work.tile([128, 3], F32, tag="nom")
        nc.vector.tensor_scalar(nom[:], r0t[:], -1.0, P_DIST, ALU.mult, ALU.add)
        nc.vector.tensor_mul(d6[:, 0:3], nom[:], inv[:])
        nc.vector.tensor_scalar(nom[:], r0t[:], -1.0, -P_DIST, ALU.mult, ALU.add)
        nc.vector.tensor_mul(d6[:, 3:6], nom[:], inv[:])

        # p_int[r, k, j] = ray0[r, j] + d6[r, k] * rd[r, j]
        pi = work.tile([128, 18], F32, tag="pi")
        nc.vector.tensor_tensor(pi[:], expand_ap(d6[:], [(1, 6), (0, 3)]),
                                expand_ap(rdt[:], [(0, 6), (1, 3)]), ALU.mult)
        nc.vector.tensor_tensor(pi[:], pi[:],
                                expand_ap(r0t[:], [(0, 6), (1, 3)]), ALU.add)
        nc.vector.tensor_scalar(pi[:].bitcast(U32), pi[:].bitcast(U32),
                                0x7FFFFFFF, None, ALU.bitwise_and)
        nc.vector.tensor_scalar(pi[:], pi[:], P_DIST + EPS, None, ALU.is_le)
        in6 = work.tile([128, 6], F32, tag="in6")
        nc.vector.tensor_reduce(in6[:], pi[:].rearrange("p (k j) -> p k j", j=3),
                                axis=AX.X, op=ALU.min)
        cnt = work.tile([128, 1], F32, tag="cnt")
        nc.vector.tensor_reduce(cnt[:], in6[:], axis=AX.X, op=ALU.add)
        mcube = work.tile([128, 1], F32, tag="mcube")
        nc.vector.tensor_scalar(mcube[:], cnt[:], 2.0, None, ALU.is_equal)

        # dn = min inside |d|, df = max inside |d|
        nc.vector.tensor_scalar(d6[:].bitcast(U32), d6[:].bitcast(U32),
                                0x7FFFFFFF, None, ALU.bitwise_and)
        lo6 = work.tile([128, 6], F32, tag="lo6")
        nc.vector.tensor_copy(lo6[:], bigt[:])
        nc.vector.copy_predicated(lo6[:], in6[:].bitcast(U32), d6[:])
        dn_r = work.tile([128, 1], F32, tag="dn_r")
        nc.vector.tensor_reduce(dn_r[:], lo6[:], axis=AX.X, op=ALU.min)
        hi6 = work.tile([128, 6], F32, tag="hi6")
        nc.vector.tensor_mul(hi6[:], in6[:], d6[:])
        df_r = work.tile([128, 1], F32, tag="df_r")
        nc.vector.tensor_reduce(df_r[:], hi6[:], axis=AX.X, op=ALU.max)

        # st2 = [mcube*dn, mcube*(df-dn-2.4)+2.4]
        st2 = work.tile([128, 2], F32, tag="st2")
        g_r = work.tile([128, 1], F32, tag="g_r")
        nc.vector.tensor_sub(g_r[:], df_r[:], dn_r[:])
        nc.vector.tensor_mul(st2[:, 0:1], mcube[:], dn_r[:])
        nc.vector.tensor_scalar(g_r[:], g_r[:], 1.0, -(DEPTH_HI - DEPTH_LO),
                                ALU.mult, ALU.add)
        nc.vector.tensor_mul(g_r[:], g_r[:], mcube[:])
        nc.vector.tensor_scalar(st2[:, 1:2], g_r[:], 1.0, DEPTH_HI - DEPTH_LO,
                                ALU.mult, ALU.add)

        pst = psZ4.tile([128, BP], F32, tag="z4")
        nc.tensor.transpose(pst[0:2, 0:128], st2[:], ident[:])
        strow = work.tile([2, 128], F32, tag="strow")
        nc.vector.tensor_copy(strow[:], pst[0:2, 0:128])
        nc.sync.dma_start(out=statsd[p, :, :], in_=strow[:])

    # (tile,k)-partition layouts for the proposal-depth construction
    dn128 = sing.tile([128, NTILE], F32)
    g128 = sing.tile([128, NTILE], F32)
    for p in range(NTILE):
        nc.sync.dma_start(out=dn128[p * 16:(p + 1) * 16, :],
                          in_=statsd[p, 0, :].rearrange("(k r) -> k r", r=8))
        nc.sync.dma_start(out=g128[p * 16:(p + 1) * 16, :],
                          in_=statsd[p, 1, :].rearrange("(k r) -> k r", r=8))
    # (ray,tile) layouts for the scan / secant math
    st_dnc = sing.tile([128, NTILE], F32)
    st_gc = sing.tile([128, NTILE], F32)
    nc.sync.dma_start(out=st_dnc[:], in_=statsd[:, 0, :].rearrange("p r -> r p"))
    nc.sync.dma_start(out=st_gc[:], in_=statsd[:, 1, :].rearrange("p r -> r p"))


    # ================= proposal depths D8b =================
    # D8b[(p,k), (r8, s)] = dn128[(p,k), r8] + t[s] * g128[(p,k), r8]
    D8b = sing.tile([128, BP], F32)
    nc.vector.tensor_tensor(D8b[:], expand_ap(t128[:], [(0, 8), (1, S)]),
                            expand_ap(g128[:], [(1, 8), (0, S)]), ALU.mult)
    nc.vector.tensor_tensor(D8b[:], D8b[:],
                            expand_ap(dn128[:], [(1, 8), (0, S)]), ALU.add)

    vald = dram.tile([NB, BP], F32)     # stage-A logits, flat batch rows
    d8d = dram.tile([NB, BP], F32)      # proposal depths, flat batch rows
    nc.sync.dma_start(out=d8d[:], in_=D8b[:])

    # ================= MLP forward for one 1024-point batch ==============
    def _mlp_halved(z1, db_ap, want_sig):
        """1024-point MLP in two 512-wide halves, emitted layer-interleaved
        so the serial latency chain of a secant eval is ~halved."""
        e1 = ep.tile([128, 2 * BP], F32, tag="e", name="e1", bufs=3)
        h1 = ep.tile([128, 2 * BP], F32R, tag="h", name="h1", bufs=4)
        e2 = ep.tile([128, 2 * BP], F32, tag="e", name="e2", bufs=3)
        h2 = ep.tile([128, 2 * BP], F32R, tag="h", name="h2", bufs=4)
        e3 = ep.tile([128, 2 * BP], F32, tag="e", name="e3", bufs=3)
        h3 = ep.tile([128, 2 * BP], F32R, tag="h", name="h3", bufs=4)
        z4 = psZ4.tile([128, BP], F32, tag="z4", name="z4")
        sigs = []
        if want_sig:
            sigs = [fin.tile([128, 2 * BP], F32, tag="s",
                             name=f"sg{i}", bufs=3) for i in (1, 2, 3)]

        def half(ap2, hh):
            # strided view covering both 128-feature chunks of one half
            return bass.AP(tensor=ap2.tensor, offset=ap2.offset + hh * 512,
                           ap=[ap2.ap[0], [BP, 2], [1, 512]])

        def fma_h(hh):
            for c in range(2):
                sl = slice(c * BP + hh * 512, c * BP + (hh + 1) * 512)
                nc.vector.tensor_mul(z1[:, sl],
                                     db_ap[:, hh * 512:(hh + 1) * 512],
                                     C[c][:, hh * 512:(hh + 1) * 512])
                nc.vector.tensor_add(z1[:, sl], z1[:, sl],
                                     A[c][:, hh * 512:(hh + 1) * 512])

        def l1_h(hh):
            nc.scalar.activation(half(e1[:], hh), half(z1[:], hh), AF.Exp)
            if want_sig:
                nc.scalar.activation(half(sigs[0][:], hh), half(z1[:], hh),
                                     AF.Sigmoid)
            nc.scalar.activation(half(h1[:], hh), half(e1[:], hh),
                                 AF.Ln, bias=1.0)

        def mm_h(h_in, wk, bk, e, sl_sig, h_out, hh):
            for c in range(2):
                ps = psA.tile([128, BP], F32, tag="z", name="psb")
                for k in range(2):
                    nc.tensor.matmul(
                        ps[:, 0:512], wk[k][c][:],
                        h_in[:, k * BP + hh * 512:k * BP + (hh + 1) * 512],
                        start=(k == 0), stop=(k == 1))
                csl = slice(c * BP + hh * 512, c * BP + (hh + 1) * 512)
                nc.scalar.activation(e[:, csl], ps[:, 0:512],
                                     AF.Exp, bias=bk[c][:])
                if sl_sig is not None:
                    nc.scalar.activation(sl_sig[:, csl], ps[:, 0:512],
                                         AF.Sigmoid, bias=bk[c][:])
            nc.scalar.activation(half(h_out[:], hh), half(e[:], hh),
                                 AF.Ln, bias=1.0)

        def z4_h(hh):
            for k in range(2):
                nc.tensor.matmul(
                    z4[0:1, hh * 512:(hh + 1) * 512], w4[k][:],
                    h3[:, k * BP + hh * 512:k * BP + (hh + 1) * 512],
                    start=(k == 0), stop=(k == 1))

        sg = (lambda i: sigs[i] if want_sig else None)
        stages = [fma_h, l1_h,
                  lambda hh: mm_h(h1, w2, b2c, e2, sg(1), h2, hh),
                  lambda hh: mm_h(h2, w3, b3c, e3, sg(2), h3, hh),
                  z4_h]
        for step in range(len(stages) + 1):
            if step < len(stages):
                stages[step](0)
            if step >= 1:
                stages[step - 1](1)
        return z4, (sigs if want_sig else None)

    def mlp_batch(db_ap, per_ray, want_sig, tag_final=False):
        """db_ap: [128, 1024] depth broadcast. per_ray=(p0,k0) selects
        stage-A per-ray scalars; None means point index == ray index.
        Returns (z4 psum tile, e_list or None)."""
        z1 = z1p.tile([128, 2 * BP], F32, tag="z1")
        if per_ray is not None:
            p0, k0 = per_ray
            g0 = p0 * 128 + k0 * 8
            for c in range(2):
                for j in range(8):
                    nc.vector.tensor_scalar(
                        z1[:, c * BP + j * 128:c * BP + (j + 1) * 128],
                        db_ap[:, j * 128:(j + 1) * 128],
                        C[c][:, g0 + j:g0 + j + 1],
                        A[c][:, g0 + j:g0 + j + 1],
                        ALU.mult, ALU.add)
        else:
            return _mlp_halved(z1, db_ap, want_sig)

        e1 = ep.tile([128, 2 * BP], F32, tag="e", name="e1", bufs=3)
        nc.scalar.activation(e1[:], z1[:], AF.Exp)
        sigs = []
        if want_sig:
            s1 = fin.tile([128, 2 * BP], F32, tag="s", name="s1", bufs=3)
            nc.scalar.activation(s1[:], z1[:], AF.Sigmoid)
            sigs.append(s1)
        h1 = ep.tile([128, 2 * BP], F32R, tag="h", name="h1", bufs=4)
        nc.scalar.activation(h1[:], e1[:], AF.Ln, bias=1.0)

        def layer(h_in, wk, bk, tagl):
            e = ep.tile([128, 2 * BP], F32, tag="e", name="e" + tagl, bufs=3)
            if want_sig:
                sl = fin.tile([128, 2 * BP], F32, tag="s", name="s" + tagl, bufs=3)
            for c in range(2):
                ps = psA.tile([128, BP], F32, tag="z")
                for k in range(2):
                    for hh in range(2):
                        nc.tensor.matmul(
                            ps[:, hh * 512:(hh + 1) * 512], wk[k][c][:],
                            h_in[:, k * BP + hh * 512:k * BP + (hh + 1) * 512],
                            start=(k == 0), stop=(k == 1))
                nc.scalar.activation(e[:, c * BP:(c + 1) * BP], ps[:],
                                     AF.Exp, bias=bk[c][:])
                if want_sig:
                    nc.scalar.activation(sl[:, c * BP:(c + 1) * BP], ps[:],
                                         AF.Sigmoid, bias=bk[c][:])
            if want_sig:
                sigs.append(sl)
            h = ep.tile([128, 2 * BP], F32R, tag="h", name="h" + tagl, bufs=4)
            nc.scalar.activation(h[:], e[:], AF.Ln, bias=1.0)
            return h

        h2 = layer(h1, w2, b2c, "2")
        h3 = layer(h2, w3, b3c, "3")
        z4 = psZ4.tile([128, BP], F32, tag="z4")
        for k in range(2):
            for hh in range(2):
                nc.tensor.matmul(z4[0:1, hh * 512:(hh + 1) * 512], w4[k][:],
                                 h3[:, k * BP + hh * 512:k * BP + (hh + 1) * 512],
                                 start=(k == 0), stop=(k == 1))
        return z4, (sigs if want_sig else None)

    def z4_to_dram(z4, dst_row_ap):
        row = rowp.tile([1, BP], F32, tag="z4row", name="row")
        nc.vector.tensor_copy(row[:], z4[0:1, :])
        nc.sync.dma_start(out=dst_row_ap, in_=row[:])

    # ================= stage A (3-deep software pipeline) =================
    # front(b): depth bcast + layer-1 FMA + Exp/Ln -> h1
    # mid(b):   layer-2 matmuls + Exp/Ln -> h2
    # back(b):  layer-3 matmuls + Exp/Ln -> h3, z4 matmuls, row copy to DRAM
    live = {}

    def stage_front(b):
        p0, k0 = b // 16, b % 16
        g0 = p0 * 128 + k0 * 8
        drow = rowp.tile([1, BP], F32, tag="drow", name="drow")
        nc.sync.dma_start(out=drow[:], in_=d8d[b:b + 1, :])
        db = bcp.tile([128, BP], F32, tag="db", name="db")
        nc.gpsimd.partition_broadcast(db[:], drow[:])
        z1 = z1p.tile([128, 2 * BP], F32, tag="z1", name="z1")
        for c in range(2):
            for j in range(8):
                nc.vector.tensor_scalar(
                    z1[:, c * BP + j * 128:c * BP + (j + 1) * 128],
                    db[:, j * 128:(j + 1) * 128],
                    C[c][:, g0 + j:g0 + j + 1],
                    A[c][:, g0 + j:g0 + j + 1],
                    ALU.mult, ALU.add)
        e1 = ep.tile([128, 2 * BP], F32, tag="e", name="e1", bufs=3)
        nc.scalar.activation(e1[:], z1[:], AF.Exp)
        h1 = ep.tile([128, 2 * BP], F32R, tag="h", name="h1", bufs=4)
        nc.scalar.activation(h1[:], e1[:], AF.Ln, bias=1.0)
        live[b] = {"h1": h1}

    def _mm_layer(h_in, wk, bk, nm):
        e = ep.tile([128, 2 * BP], F32, tag="e", name="e" + nm, bufs=3)
        for c in range(2):
            ps = psA.tile([128, BP], F32, tag="z", name="ps" + nm)
            for k in range(2):
                for hh in range(2):
                    nc.tensor.matmul(
                        ps[:, hh * 512:(hh + 1) * 512], wk[k][c][:],
                        h_in[:, k * BP + hh * 512:k * BP + (hh + 1) * 512],
                        start=(k == 0), stop=(k == 1))
            nc.scalar.activation(e[:, c * BP:(c + 1) * BP], ps[:],
                                 AF.Exp, bias=bk[c][:])
        h = ep.tile([128, 2 * BP], F32R, tag="h", name="h" + nm, bufs=4)
        nc.scalar.activation(h[:], e[:], AF.Ln, bias=1.0)
        return h

    def stage_mid(b):
        live[b]["h2"] = _mm_layer(live[b].pop("h1"), w2, b2c, "2")

    def stage_back(b):
        h3 = _mm_layer(live[b].pop("h2"), w3, b3c, "3")
        z4 = psZ4.tile([128, BP], F32, tag="z4", name="z4A")
        for k in range(2):
            for hh in range(2):
                nc.tensor.matmul(z4[0:1, hh * 512:(hh + 1) * 512], w4[k][:],
                                 h3[:, k * BP + hh * 512:k * BP + (hh + 1) * 512],
                                 start=(k == 0), stop=(k == 1))
        z4_to_dram(z4, vald[b:b + 1, :])
        del live[b]

    for _rep in range(REPS):
        for i in range(NB + 2):
            if i < NB:
                stage_front(i)
            if 1 <= i and i - 1 < NB:
                stage_mid(i - 1)
            if 2 <= i and i - 2 < NB:
                stage_back(i - 2)


    # ================= scan =================
    st_dlow = sing.tile([128, NTILE], F32)
    st_flow = sing.tile([128, NTILE], F32)
    st_dhigh = sing.tile([128, NTILE], F32)
    st_fhigh = sing.tile([128, NTILE], F32)
    st_mask = sing.tile([128, NTILE], F32)

    for p in range(NTILE):
        val = work.tile([128, S], F32, tag="val")
        nc.sync.dma_start(
            out=val[:],
            in_=vald[p * 16:(p + 1) * 16, :].rearrange(
                "k (r s) -> (k r) s", s=S))
        nc.vector.tensor_scalar(val[:], val[:], b4col[:], -LOGIT_TAU,
                                ALU.add, ALU.add)

        prod = work.tile([128, S], F32, tag="prod")
        nc.vector.tensor_tensor(prod[:, 0:S - 1], val[:, 0:S - 1],
                                val[:, 1:S], ALU.mult)
        sgn = work.tile([128, S], F32, tag="sgn")
        nc.scalar.activation(sgn[:, 0:S - 1], prod[:, 0:S - 1], AF.Sign)
        nc.vector.memset(sgn[:, S - 1:S], 1.0)
        cost = work.tile([128, S], F32, tag="cost")
        nc.vector.tensor_tensor(cost[:], sgn[:], wcost[:], ALU.mult)
        vmin = work.tile([128, 1], F32, tag="vmin")
        nc.vector.tensor_reduce(vmin[:], cost[:], axis=AX.X, op=ALU.min)
        oh = work.tile([128, S], F32, tag="oh")
        nc.vector.tensor_scalar(oh[:], cost[:], vmin[:], None, ALU.is_equal)
        ohh = work.tile([128, S], F32, tag="ohh")
        nc.vector.memset(ohh[:, 0:1], 0.0)
        nc.vector.tensor_copy(ohh[:, 1:S], oh[:, 0:S - 1])
        nc.vector.tensor_add(ohh[:, S - 1:S], ohh[:, S - 1:S], oh[:, S - 1:S])

        scr = work.tile([128, S], F32, tag="scr")
        f_low = work.tile([128, 1], F32, tag="f_low")
        f_high = work.tile([128, 1], F32, tag="f_high")
        t_low = work.tile([128, 1], F32, tag="t_low")
        t_high = work.tile([128, 1], F32, tag="t_high")
        nc.vector.tensor_mul(scr[:], val[:], oh[:])
        nc.vector.tensor_reduce(f_low[:], scr[:], axis=AX.X, op=ALU.add)
        nc.vector.tensor_mul(scr[:], val[:], ohh[:])
        nc.vector.tensor_reduce(f_high[:], scr[:], axis=AX.X, op=ALU.add)
        nc.vector.tensor_mul(scr[:], t128[:], oh[:])
        nc.vector.tensor_reduce(t_low[:], scr[:], axis=AX.X, op=ALU.add)
        nc.vector.tensor_mul(scr[:], t128[:], ohh[:])
        nc.vector.tensor_reduce(t_high[:], scr[:], axis=AX.X, op=ALU.add)

        nc.vector.tensor_mul(t_low[:], t_low[:], st_gc[:, p:p + 1])
        nc.vector.tensor_add(st_dlow[:, p:p + 1], t_low[:], st_dnc[:, p:p + 1])
        nc.vector.tensor_mul(t_high[:], t_high[:], st_gc[:, p:p + 1])
        nc.vector.tensor_add(st_dhigh[:, p:p + 1], t_high[:],
                             st_dnc[:, p:p + 1])
        nc.vector.tensor_copy(st_flow[:, p:p + 1], f_low[:])
        nc.vector.tensor_copy(st_fhigh[:, p:p + 1], f_high[:])

        m1 = work.tile([128, 1], F32, tag="m1")
        nc.vector.tensor_scalar(m1[:], vmin[:], 0.0, None, ALU.is_lt)
        m2 = work.tile([128, 1], F32, tag="m2")
        nc.vector.tensor_scalar(m2[:], f_low[:], 0.0, None, ALU.is_lt)
        nc.vector.tensor_mul(m1[:], m1[:], m2[:])
        nc.vector.tensor_scalar(m2[:], val[:, 0:1], 0.0, None, ALU.is_lt)
        nc.vector.tensor_mul(st_mask[:, p:p + 1], m1[:], m2[:])


    # ================= secant =================
    st_dpred = sing.tile([128, NTILE], F32)

    def secant_dpred():
        den = work.tile([128, NTILE], F32, tag="sden")
        nc.vector.tensor_sub(den[:], st_fhigh[:], st_flow[:])
        dabs = work.tile([128, NTILE], F32, tag="sdabs")
        nc.vector.tensor_scalar(dabs[:].bitcast(U32), den[:].bitcast(U32),
                                0x7FFFFFFF, None, ALU.bitwise_and)
        msk = work.tile([128, NTILE], F32, tag="smsk")
        nc.vector.tensor_scalar(msk[:], dabs[:], 1e-12, None, ALU.is_lt)
        nc.vector.copy_predicated(den[:], msk[:].bitcast(U32), c1em12[:])
        rec = work.tile([128, NTILE], F32, tag="srec")
        nc.vector.reciprocal(rec[:], den[:])
        num = work.tile([128, NTILE], F32, tag="snum")
        nc.vector.tensor_sub(num[:], st_dhigh[:], st_dlow[:])
        nc.vector.tensor_mul(num[:], num[:], st_flow[:])
        nc.vector.tensor_mul(num[:], num[:], rec[:])
        nc.vector.tensor_sub(st_dpred[:], st_dlow[:], num[:])

    secant_dpred()

    dpredd = dram.tile([NSEC + 1, R], F32)   # flattened d_pred per use
    fmd = dram.tile([NSEC + 1, R], F32)      # flattened f / df rows

    def eval_f(i, want_sig=False, tag_final=False):
        nc.sync.dma_start(out=dpredd[i, :].rearrange("(t r) -> r t", r=128),
                          in_=st_dpred[:])
        dfl = rowp.tile([1, R], F32, tag="dflat", name="dfl")
        nc.sync.dma_start(out=dfl[:], in_=dpredd[i:i + 1, :])
        db = bcp.tile([128, R], F32, tag="db")
        nc.gpsimd.partition_broadcast(db[:], dfl[:])
        z4, es = mlp_batch(db[:], None, want_sig, tag_final)
        z4_to_dram(z4, fmd[i:i + 1, :])
        fm = work.tile([128, NTILE], F32, tag="fm")
        nc.sync.dma_start(out=fm[:],
                          in_=fmd[i, :].rearrange("(t r) -> r t", r=128))
        nc.vector.tensor_scalar(fm[:], fm[:], b4col[:], -LOGIT_TAU,
                                ALU.add, ALU.add)
        return fm, db, es

    for it in range(NSEC):
        fm, _, _ = eval_f(it)
        low = work.tile([128, NTILE], F32, tag="lowm")
        nc.vector.tensor_scalar(low[:], fm[:], 0.0, None, ALU.is_lt)
        nc.vector.copy_predicated(st_dlow[:], low[:].bitcast(U32), st_dpred[:])
        nc.vector.copy_predicated(st_flow[:], low[:].bitcast(U32), fm[:])
        hi = work.tile([128, NTILE], F32, tag="him")
        nc.vector.tensor_scalar(hi[:], low[:], -1.0, 1.0, ALU.mult, ALU.add)
        nc.vector.copy_predicated(st_dhigh[:], hi[:].bitcast(U32), st_dpred[:])
        nc.vector.copy_predicated(st_fhigh[:], hi[:].bitcast(U32), fm[:])
        secant_dpred()


    # ================= final newton correction =================
    fval, dbF, sigs = eval_f(NSEC, want_sig=True, tag_final=True)
    s1, s2, s3 = sigs

    dh1 = fin.tile([128, 2 * BP], F32R, tag="dh", name="dh1", bufs=2)
    for c in range(2):
        nc.vector.tensor_mul(dh1[:, c * BP:(c + 1) * BP],
                             s1[:, c * BP:(c + 1) * BP], C[c][:])

    def tangent_layer(dh_in, wk, s_l, tagl):
        dh = fin.tile([128, 2 * BP], F32R, tag="dh", name="dh", bufs=2)
        for c in range(2):
            ps = psA.tile([128, BP], F32, tag="z")
            for k in range(2):
                for hh in range(2):
                    nc.tensor.matmul(
                        ps[:, hh * 512:(hh + 1) * 512], wk[k][c][:],
                        dh_in[:, k * BP + hh * 512:k * BP + (hh + 1) * 512],
                        start=(k == 0), stop=(k == 1))
            nc.vector.tensor_mul(dh[:, c * BP:(c + 1) * BP],
                                 s_l[:, c * BP:(c + 1) * BP], ps[:])
        return dh

    dh2 = tangent_layer(dh1, w2, s2, "2")
    dh3 = tangent_layer(dh2, w3, s3, "3")
    dfp = psZ4.tile([128, BP], F32, tag="z4")
    for k in range(2):
        for hh in range(2):
            nc.tensor.matmul(dfp[0:1, hh * 512:(hh + 1) * 512], w4[k][:],
                             dh3[:, k * BP + hh * 512:k * BP + (hh + 1) * 512],
                             start=(k == 0), stop=(k == 1))
    dfdd_d = dram.tile([1, R], F32)
    row = rowp.tile([1, BP], F32, tag="z4row", name="row")
    nc.vector.tensor_copy(row[:], dfp[0:1, :])
    nc.sync.dma_start(out=dfdd_d[:], in_=row[:])
    dfdd = work.tile([128, NTILE], F32, tag="dfdd")
    nc.sync.dma_start(out=dfdd[:],
                      in_=dfdd_d[0, :].rearrange("(t r) -> r t", r=128))

    # clamp: |df|<1e-6 -> sign(df)*1e-6 (df==0 -> +1e-6)
    dneg = work.tile([128, NTILE], F32, tag="dneg")
    nc.vector.tensor_scalar(dneg[:], dfdd[:], 0.0, None, ALU.is_lt)
    dabs = work.tile([128, NTILE], F32, tag="dfabs")
    nc.vector.tensor_scalar(dabs[:].bitcast(U32), dfdd[:].bitcast(U32),
                            0x7FFFFFFF, None, ALU.bitwise_and)
    nc.vector.tensor_scalar(dabs[:], dabs[:], 1e-6, None, ALU.max)
    nc.vector.tensor_scalar(dneg[:], dneg[:], -2.0, 1.0, ALU.mult, ALU.add)
    nc.vector.tensor_mul(dfdd[:], dabs[:], dneg[:])

    rec = work.tile([128, NTILE], F32, tag="recF")
    nc.vector.reciprocal(rec[:], dfdd[:])
    nc.vector.tensor_mul(fval[:], fval[:], rec[:])
    dout = work.tile([128, NTILE], F32, tag="dout")
    nc.vector.tensor_sub(dout[:], st_dpred[:], fval[:])
    nc.vector.tensor_mul(dout[:], dout[:], st_mask[:])

    nc.sync.dma_start(out=d_out.rearrange("(t r) -> r t", r=128), in_=dout[:])

    ctx.close()


# ======================= host-side driver =======================

def _host_constants():
    t = np.linspace(0.0, 1.0, S).astype(np.float32)
    t128 = np.ascontiguousarray(np.broadcast_to(t, (128, S)))
    w = np.arange(S, 0, -1, dtype=np.float32)   # S, S-1, ..., 1
    wcost = np.ascontiguousarray(np.broadcast_to(w, (128, S)))
    ident = np.eye(128, dtype=np.float32)
    return t128, wcost, ident


def _make_runner(nc):
    """Persistent 8-core PJRT runner (mirrors bass2jax.run_bass_via_pjrt's
    multi-core path, but keeps the compiled shard_map callable across calls)."""
    import jax
    from jax.sharding import Mesh, PartitionSpec
    from jax.experimental.shard_map import shard_map
    from concourse import bass2jax
    from concourse import mybir as _mb

    bass2jax.install_neuronx_cc_hook()
    partition_name = (nc.partition_id_tensor.name
                      if nc.partition_id_tensor else None)
    in_names, out_names, out_avals, zero_shapes = [], [], [], []
    for alloc in nc.m.functions[0].allocations:
        if not isinstance(alloc, _mb.MemoryLocationSet):
            continue
        name = alloc.memorylocations[0].name
        if alloc.kind == "ExternalInput":
            if name != partition_name:
                in_names.append(name)
        elif alloc.kind == "ExternalOutput":
            out_names.append(name)
            shape = tuple(alloc.tensor_shape)
            dtype = _mb.dt.np(alloc.dtype)
            out_avals.append(jax.core.ShapedArray(shape, dtype))
            zero_shapes.append((shape, dtype))
    n_params = len(in_names)
    n_outs = len(out_avals)
    all_names = list(in_names) + list(out_names)
    if partition_name is not None:
        all_names.append(partition_name)
    donate = tuple(range(n_params, n_params + n_outs))

    def _body(*args):
        operands = list(args)
        if partition_name is not None:
            operands.append(bass2jax.partition_id_tensor())
        outs = bass2jax._bass_exec_p.bind(
            *operands,
            out_avals=tuple(out_avals),
            in_names=tuple(all_names),
            out_names=tuple(out_names),
            lowering_input_output_aliases=(),
            sim_require_finite=True,
            sim_require_nnan=True,
            nc=nc,
        )
        return tuple(outs)

    devices = jax.devices()[:NCORES]
    mesh = Mesh(np.asarray(devices), ("core",))
    in_specs = (PartitionSpec("core"),) * (n_params + n_outs)
    out_specs = (PartitionSpec("core"),) * n_outs
    sharded = jax.jit(
        shard_map(_body, mesh=mesh, in_specs=in_specs, out_specs=out_specs,
                  check_rep=False),
        donate_argnums=donate, keep_unused=True)

    def run(in_maps):
        concat_in = [
            np.concatenate([np.asarray(in_maps[c][nm]) for c in range(NCORES)],
                           axis=0)
            for nm in in_names
        ]
        concat_zeros = [np.zeros((NCORES * sh[0], *sh[1:]), dt)
                        for (sh, dt) in zero_shapes]
        outs = jax.block_until_ready(sharded(*concat_in, *concat_zeros))
        return [
            {nm: np.asarray(outs[i]).reshape(NCORES, *out_avals[i].shape)[c]
             for i, nm in enumerate(out_names)}
            for c in range(NCORES)
        ]

    return run


def kernel(ray0, ray_direction, W1, b1, W2, b2, W3, b3, W4, b4):
    if "prog" not in _nc_cache:
        _nc_cache["prog"] = build_program()
        _nc_cache["runner"] = _make_runner(_nc_cache["prog"])
    nc = _nc_cache["prog"]

    t128, wcost, ident = _host_constants()
    r0 = np.ascontiguousarray(np.asarray(ray0).reshape(N, 3), np.float32)
    rd = np.ascontiguousarray(np.asarray(ray_direction).reshape(N, 3),
                              np.float32)
    shared = {
        "W1": np.ascontiguousarray(W1, np.float32),
        "b1": np.ascontiguousarray(b1, np.float32),
        "W2": np.ascontiguousarray(W2, np.float32),
        "b2": np.ascontiguousarray(b2, np.float32),
        "W3": np.ascontiguousarray(W3, np.float32),
        "b3": np.ascontiguousarray(b3, np.float32),
        "W4": np.ascontiguousarray(W4, np.float32),
        "b4": np.ascontiguousarray(b4, np.float32),
        "t128": t128, "wcost": wcost, "ident": ident,
    }
    in_maps = []
    for c in range(NCORES):
        m = dict(shared)
        m["ray0"] = r0[c * R:(c + 1) * R]
        m["rd"] = rd[c * R:(c + 1) * R]
        in_maps.append(m)

    results = _nc_cache["runner"](in_maps)
    out = np.concatenate([results[c]["out"] for c in range(NCORES)])
    return out.reshape(1, N).astype(np.float32)
